# revision 6
# baseline (speedup 1.0000x reference)
"""Trainium2 Bass kernel for nn_EnhancedGCN42 (4-layer GCN + MLP classifier).

Strategy (8 NeuronCores, SPMD single NEFF):
  - Nodes dst-sharded: device d owns dst nodes [d*12500, (d+1)*12500).
  - A-hat = D^-1/2 (A+I) D^-1/2 factorized: tables store dis*h rows (bf16,
    256B rows); aggregation output scaled by dis_dst.
  - Row space is chunk-major with uneven chunks (16,16,16,16,16,16,2 tiles
    per device). Each phase's table lives in 7 separate DRAM tensors (one
    per chunk) so the AllGather of chunk c and the gathers that read it
    have exact deps: chunk c's AllGather is issued on the sync engine
    right after its block's epilogue writes (so it never blocks the
    gpsimd gather-dispatch stream), and next-phase gathers for range r
    wait only on chunk r. The tiny last chunk minimizes the refill
    bubble at phase boundaries.
  - Per layer: per-edge rows gathered via dma_gather (4 SWDGE queues),
    aggregated per 128-dst tile by matmul with an on-chip-built one-hot
    selection matrix (per-use vector tensor_scalar is_equal, 4x DVE).
  - Dense W / BN / ReLU fused per dst-tile in transposed layout; BN and
    classifier BN folded on host into per-feature scale/bias.

kernel(**inputs) -> [100000, 2] float32.
"""
import hashlib
import numpy as np
import ml_dtypes

import concourse.bacc as bacc
import concourse.bass as bass
import concourse.mybir as mybir
import concourse.tile as tile
from concourse.masks import make_identity
from concourse.bass_utils import run_bass_kernel_spmd

f32 = mybir.dt.float32
bf16 = mybir.dt.bfloat16
i16 = mybir.dt.int16
i32 = mybir.dt.int32
nbf16 = ml_dtypes.bfloat16

P = 128
NDEV = 8
EPS = 1e-5
WTAB = 128       # table row = 128 cols bf16 = 256B
CH_TILES = [16, 16, 16, 16, 16, 14, 4]   # tiles per chunk (sum = NT = 98)
NR = len(CH_TILES)


def _prep(x, edge_index, params, N):
    """Host preprocessing: graph partition + folded constants. Returns meta dict."""
    SHARD = N // NDEV                       # 12500
    TSHARD = ((SHARD + P - 1) // P) * P     # 12544
    NT = TSHARD // P                        # 98
    TROWS = TSHARD * NDEV                   # 100352
    assert sum(CH_TILES) == NT
    ch_t0 = np.cumsum([0] + CH_TILES)                    # tile offsets [NR+1]
    csh = [ct * P for ct in CH_TILES]                    # rows/device/chunk
    rngw = [c * NDEV for c in csh]                       # rows/chunk (global)
    rng0 = np.cumsum([0] + rngw)                         # chunk row offsets
    assert all(w <= 32768 for w in rngw)

    ei = edge_index.astype(np.int64)
    loop = np.arange(N, dtype=np.int64)
    dst_all = np.concatenate([ei[1], loop])
    deg = np.bincount(dst_all, minlength=N).astype(np.float32)
    dis = (1.0 / np.sqrt(deg)).astype(np.float32)

    def padrow(n):
        # node -> chunk-major padded row. local tile lt = local//128 belongs
        # to chunk c(lt); row = rng0[c] + d*csh[c] + (local - ch_t0[c]*128)
        d = n // SHARD
        local = n - d * SHARD
        lt = local // P
        c = np.searchsorted(ch_t0, lt // 1 + 1) - 1  # chunk of tile lt
        c = np.minimum(c, NR - 1)
        return rng0[c] + d * np.array(csh)[c] + (local - ch_t0[c] * P)

    src_e = ei[0]
    dst_e = ei[1]
    psrc_e = padrow(src_e)
    r_of_row = lambda rows: np.searchsorted(rng0, rows, side="right") - 1

    counts = np.zeros((NDEV, NT, NR), dtype=np.int64)
    dev_edges = []
    for d in range(NDEV):
        m = (dst_e >= d * SHARD) & (dst_e < (d + 1) * SHARD)
        es = psrc_e[m]
        el = dst_e[m] - d * SHARD
        t_id = el >> 7
        r_id = r_of_row(es)
        order = np.lexsort((es, r_id, t_id))  # (tile, range, src-ascending)
        es, el, t_id, r_id = es[order], el[order], t_id[order], r_id[order]
        np.add.at(counts[d], (t_id, r_id), 1)
        dev_edges.append((es, el, r_id))

    grp_rows = counts.max(axis=0).astype(np.int64)  # [NT, NR] exact max

    n_blk = NR  # gather blocks == chunks (uneven tile counts)
    grp_off = np.zeros((NT, NR), dtype=np.int64)
    blk_off = np.zeros((n_blk, NR), dtype=np.int64)
    blk_rows = np.zeros((n_blk, NR), dtype=np.int64)
    acc = 0
    for b in range(n_blk):
        for r in range(NR):
            blk_off[b, r] = acc
            for t in range(ch_t0[b], ch_t0[b + 1]):
                grp_off[t, r] = acc
                acc += grp_rows[t, r]
            acc = (acc + P - 1) // P * P  # pad gather to whole chunks
            blk_rows[b, r] = acc - blk_off[b, r]
    TOT = acc
    assert TOT % 16 == 0

    # chunk-use enumeration: per (t, r) the 128-row chunks its group overlaps.
    uses = [[[] for _ in range(NR)] for _ in range(NT)]  # (local_col, dstl_col)
    n_uses = 0
    for b in range(n_blk):
        for r in range(NR):
            for t in range(ch_t0[b], ch_t0[b + 1]):
                g0, g1 = grp_off[t, r], grp_off[t, r] + grp_rows[t, r]
                if g1 == g0:
                    continue
                c0, c1 = int(g0 // P), int((g1 + P - 1) // P)
                for ci in range(c0, c1):
                    uses[t][r].append((ci - int(blk_off[b, r]) // P, n_uses, ci))
                    n_uses += 1

    idx_w = np.zeros((NDEV, P, TOT // 16), dtype=np.int16)
    dstl_w = np.full((NDEV, P, n_uses), 255.0, dtype=np.float32)
    for d in range(NDEV):
        es, el, _r = dev_edges[d]
        IDX = np.zeros(TOT, dtype=np.int16)
        DLOC = np.full(TOT, 255.0, dtype=np.float32)
        OWNER = np.full(TOT, -1, dtype=np.int64)
        pos = 0
        for t in range(NT):
            for r in range(NR):
                c = int(counts[d, t, r])
                o = int(grp_off[t, r])
                IDX[o:o + c] = (es[pos:pos + c] - rng0[r]).astype(np.int16)
                DLOC[o:o + c] = (el[pos:pos + c] - t * P).astype(np.float32)
                OWNER[o:o + int(grp_rows[t, r])] = t
                pos += c
        idx_w[d] = np.tile(IDX.reshape(-1, 16).T, (8, 1))
        D = dstl_w[d]
        for t in range(NT):
            for r in range(NR):
                for (_lc, du, ci) in uses[t][r]:
                    rows = np.arange(ci * P, (ci + 1) * P)
                    v = np.where(OWNER[rows] == t, DLOC[rows], 255.0)
                    D[:, du] = v

    dis_pad = np.zeros(TROWS, dtype=np.float32)
    dis_pad[padrow(np.arange(N))] = dis
    dis_t = np.zeros((NDEV, P, NT), dtype=np.float32)
    for d in range(NDEV):
        for t in range(NT):
            c = int(np.searchsorted(ch_t0, t + 1) - 1)
            g0 = rng0[c] + d * csh[c] + (t - ch_t0[c]) * P
            dis_t[d, :, t] = dis_pad[g0:g0 + P]

    # x-tilde table (bf16, padded cols), chunk-major row space
    xt = np.zeros((TROWS, WTAB), dtype=nbf16)
    v = (dis[:, None] * x).astype(nbf16)
    xt[padrow(np.arange(N)), :x.shape[1]] = v

    def fold(g, be, rm, rv, b):
        k = (1.0 / np.sqrt(rv + EPS)).astype(np.float32)
        s = g * k
        t = (b - rm) * s + be
        return s.astype(np.float32), t.astype(np.float32)

    s1, t1 = fold(params["g1"], params["be1"], params["rm1"], params["rv1"], params["b1"])
    s2, t2 = fold(params["g2"], params["be2"], params["rm2"], params["rv2"], params["b2"])
    s3, t3 = fold(params["g3"], params["be3"], params["rm3"], params["rv3"], params["b3"])
    s4, t4 = fold(params["g4"], params["be4"], params["rm4"], params["rv4"], params["b4"])
    zk = (1.0 / np.sqrt(params["crv1"] + EPS)).astype(np.float32)
    cs1 = params["cg1"] * zk
    ct1 = -params["crm1"] * cs1 + params["cbe1"]
    zk = (1.0 / np.sqrt(params["crv2"] + EPS)).astype(np.float32)
    cs2 = params["cg2"] * zk
    ct2 = -params["crm2"] * cs2 + params["cbe2"]
    cW2p = (cs1[:, None] * params["cW2"]).astype(np.float32)
    cb2p = (ct1 @ params["cW2"] + params["cb2"]).astype(np.float32)
    cW3p = (cs2[:, None] * params["cW3"]).astype(np.float32)
    cb3p = (ct2 @ params["cW3"] + params["cb3"]).astype(np.float32)

    vecs = np.zeros((P, 13), dtype=np.float32)
    vecs[:, 0], vecs[:, 1] = s1, t1
    vecs[:, 2], vecs[:, 3] = s2[:128], t2[:128]
    vecs[:, 4], vecs[:, 5] = s2[128:], t2[128:]
    vecs[:, 6], vecs[:, 7] = s3, t3
    vecs[:64, 8], vecs[:64, 9] = s4, t4
    vecs[:64, 10] = params["cb1"]
    vecs[:32, 11] = cb2p
    vecs[:2, 12] = cb3p

    return dict(
        N=N, SHARD=SHARD, TSHARD=TSHARD, NT=NT, TROWS=TROWS,
        ch_t0=ch_t0, csh=csh, rngw=rngw, rng0=rng0,
        TOT=TOT, uses=uses, n_uses=n_uses,
        n_blk=n_blk, blk_off=blk_off, blk_rows=blk_rows,
        idx_w=idx_w, dstl_w=dstl_w, dis_t=dis_t, xt=xt, vecs=vecs,
        W1=params["W1"].astype(np.float32), W2=params["W2"].astype(np.float32),
        W3=np.concatenate([params["W3"][:128], params["W3"][128:]], axis=1).astype(np.float32),
        W4=params["W4"].astype(np.float32),
        cW1=params["cW1"].astype(np.float32), cW2p=cW2p, cW3p=cW3p,
        d_in=x.shape[1],
    )


def _build(meta):
    """Build the Bass program (same for all cores)."""
    NT, TSHARD, TROWS = meta["NT"], meta["TSHARD"], meta["TROWS"]
    ch_t0, csh, rngw = meta["ch_t0"], meta["csh"], meta["rngw"]
    TOT = meta["TOT"]
    uses, n_uses = meta["uses"], meta["n_uses"]
    n_blk, blk_off, blk_rows = meta["n_blk"], meta["blk_off"], meta["blk_rows"]
    D_IN = meta["d_in"]

    nc = bacc.Bacc(None, target_bir_lowering=False, num_swdge_queues=4)
    t_xt = [nc.dram_tensor(f"xt{r}", [rngw[r], WTAB], bf16, kind="ExternalInput")
            for r in range(NR)]
    t_idx = nc.dram_tensor("idx", [P, TOT // 16], i16, kind="ExternalInput")
    t_dstl = nc.dram_tensor("dstl", [P, n_uses], f32, kind="ExternalInput")
    t_xto = nc.dram_tensor("xt_own", [TSHARD, WTAB], bf16, kind="ExternalInput")
    t_dis = nc.dram_tensor("dis", [P, NT], f32, kind="ExternalInput")
    t_vecs = nc.dram_tensor("vecs", [P, 13], f32, kind="ExternalInput")
    t_W1 = nc.dram_tensor("W1", [D_IN, 128], f32, kind="ExternalInput")
    t_W2 = nc.dram_tensor("W2", [128, 256], f32, kind="ExternalInput")
    t_W3 = nc.dram_tensor("W3", [128, 256], f32, kind="ExternalInput")  # packed K-halves
    t_W4 = nc.dram_tensor("W4", [128, 64], f32, kind="ExternalInput")
    t_cW1 = nc.dram_tensor("cW1", [64, 64], f32, kind="ExternalInput")
    t_cW2 = nc.dram_tensor("cW2p", [64, 32], f32, kind="ExternalInput")
    t_cW3 = nc.dram_tensor("cW3p", [32, 2], f32, kind="ExternalInput")
    t_out = nc.dram_tensor("outT", [2, TSHARD], f32, kind="ExternalOutput")

    cc_in = [[nc.dram_tensor(f"cc_in{k}_{c}", [csh[c], WTAB], bf16) for c in range(NR)]
             for k in range(3)]
    tabs = [[nc.dram_tensor(f"tab{k}_{c}", [rngw[c], WTAB], bf16, addr_space="Shared")
             for c in range(NR)] for k in range(3)]

    qctr = [0]

    def qrr():
        qctr[0] = (qctr[0] + 1) % 4
        return qctr[0]

    with tile.TileContext(nc) as tc:
        with (
            tc.tile_pool(name="const", bufs=1) as cpool,
            tc.tile_pool(name="gp", bufs=8) as gpool,
            tc.tile_pool(name="sp", bufs=8) as spool,
            tc.tile_pool(name="pagg", bufs=2, space="PSUM") as pagg,
            tc.tile_pool(name="paux", bufs=3, space="PSUM") as paux,
            tc.tile_pool(name="ep", bufs=3) as ep,
        ):
            # ---- constants
            idx_sb = cpool.tile([P, TOT // 16], i16)
            nc.sync.dma_start(out=idx_sb[:], in_=t_idx[:])
            dstl_sb = cpool.tile([P, n_uses], f32)
            nc.sync.dma_start(out=dstl_sb[:], in_=t_dstl[:])
            dis_sb = cpool.tile([P, NT], f32)
            nc.sync.dma_start(out=dis_sb[:], in_=t_dis[:])
            vecs_sb = cpool.tile([P, 13], f32)
            nc.sync.dma_start(out=vecs_sb[:], in_=t_vecs[:])
            W1_sb = cpool.tile([D_IN, 128], f32)
            nc.sync.dma_start(out=W1_sb[:], in_=t_W1[:])
            W2_sb = cpool.tile([128, 256], f32)
            nc.sync.dma_start(out=W2_sb[:], in_=t_W2[:])
            W3_sb = cpool.tile([128, 256], f32)
            nc.sync.dma_start(out=W3_sb[:], in_=t_W3[:])
            W4_sb = cpool.tile([128, 64], f32)
            nc.sync.dma_start(out=W4_sb[:], in_=t_W4[:])
            cW1_sb = cpool.tile([64, 64], f32)
            nc.sync.dma_start(out=cW1_sb[:], in_=t_cW1[:])
            cW2_sb = cpool.tile([64, 32], f32)
            nc.sync.dma_start(out=cW2_sb[:], in_=t_cW2[:])
            cW3_sb = cpool.tile([32, 2], f32)
            nc.sync.dma_start(out=cW3_sb[:], in_=t_cW3[:])
            ident = cpool.tile([P, P], f32)
            make_identity(nc, ident[:])
            ident_bf = cpool.tile([P, P], bf16)
            nc.vector.tensor_copy(out=ident_bf[:], in_=ident[:])
            iota_i = cpool.tile([P, P], i32)
            nc.gpsimd.iota(iota_i[:], pattern=[[1, P]], base=0,
                           channel_multiplier=0)
            iota_bf = cpool.tile([P, P], bf16)
            nc.vector.tensor_copy(out=iota_bf[:], in_=iota_i[:])

            AluEq = mybir.AluOpType.is_equal
            ACTF = mybir.ActivationFunctionType

            def transpose_f32(src_sb, pdim, fdim):
                """[pdim, fdim] f32 sbuf -> [fdim, pdim] f32 sbuf (PE transpose)."""
                tp = paux.tile([fdim, pdim], f32, tag="mm")
                nc.tensor.transpose(tp[:], src_sb[:], ident[:pdim, :pdim])
                out = ep.tile([fdim, pdim], f32, tag="tps")
                nc.vector.tensor_copy(out=out[:], in_=tp[:])
                return out

            def emit_ag(k, c):
                # chunk-c table of phase k for the next phase. Emitted with a
                # ~2-block lag so its input-ready wait is already satisfied
                # when the gpsimd dispatcher reaches it (no head-of-line
                # block of the gather stream).
                nc.gpsimd.collective_compute(
                    "AllGather", mybir.AluOpType.bypass,
                    replica_groups=[list(range(NDEV))],
                    ins=[cc_in[k][c][:]], outs=[tabs[k][c][:]],
                )

            def phase(k, w, epilogue):
                """Block-merged gathers + per-tile S-matmul aggregation."""
                if k > 0:
                    emit_ag(k - 1, 5)
                for b in range(n_blk):
                    tiles = range(ch_t0[b], ch_t0[b + 1])
                    gt = {}
                    for r in range(NR):
                        if k > 0 and b == 0 and r == 5:
                            emit_ag(k - 1, 6)
                        rows = int(blk_rows[b, r])
                        if rows == 0:
                            continue
                        g = gpool.tile([P, rows // P, WTAB], bf16, tag="g")
                        off = int(blk_off[b, r])
                        table_ap = (t_xt[r] if k == 0 else tabs[k - 1][r])
                        nc.gpsimd.dma_gather(
                            out_ap=g[:],
                            in_ap=table_ap[:, :],
                            idxs_ap=idx_sb[:, off // 16:(off + rows) // 16],
                            num_idxs=rows,
                            num_idxs_reg=rows,
                            elem_size=WTAB,
                            single_packet=False,
                            queue_num=qrr(),
                        )
                        gt[r] = g
                    if k < 3 and 0 <= b - 2 <= 4:
                        emit_ag(k, b - 2)
                    for t in tiles:
                        own = ep.tile([P, WTAB], bf16, tag="own")
                        if k == 0:
                            src_ap = t_xto[t * P:(t + 1) * P, :]
                        else:
                            c = int(np.searchsorted(ch_t0, t + 1) - 1)
                            pt = t - ch_t0[c]
                            src_ap = cc_in[k - 1][c][pt * P:(pt + 1) * P, :]
                        nc.sync.dma_start(out=own[:], in_=src_ap)
                        spt = {}
                        for r in range(NR):
                            ul = uses[t][r]
                            if not ul:
                                continue
                            sP = spool.tile([P, len(ul), P], bf16, tag="s")
                            for ui, (_lc, du, _ci) in enumerate(ul):
                                nc.vector.tensor_scalar(
                                    out=sP[:, ui, :],
                                    in0=iota_bf[:],
                                    scalar1=dstl_sb[:, du:du + 1],
                                    scalar2=None,
                                    op0=AluEq,
                                )
                            spt[r] = sP
                        nmm = 1 + sum(len(uses[t][r]) for r in range(NR))
                        ps = pagg.tile([P, w], f32, tag="pagg")
                        nc.tensor.matmul(ps[:], lhsT=ident_bf[:], rhs=own[:, :w],
                                         start=True, stop=(nmm == 1))
                        kk = 1
                        for r in range(NR):
                            for ui, (lc, du, _ci) in enumerate(uses[t][r]):
                                nc.tensor.matmul(
                                    ps[:], lhsT=spt[r][:, ui, :], rhs=gt[r][:, lc, :w],
                                    start=False, stop=(kk == nmm - 1),
                                )
                                kk += 1
                        epilogue(t, ps)

            def wr_cc(k, t, src):
                c = int(np.searchsorted(ch_t0, t + 1) - 1)
                pt = t - ch_t0[c]
                nc.sync.dma_start(out=cc_in[k][c][pt * P:(pt + 1) * P, :], in_=src)

            # ================= Phase 1: L1 =================
            def ep1(t, ps):
                a = ep.tile([P, D_IN], f32, tag="a1")
                nc.scalar.activation(a[:], ps[:], ACTF.Copy, scale=dis_sb[:, t:t + 1])
                aT = transpose_f32(a, P, D_IN)
                hps = paux.tile([128, P], f32, tag="mm")
                nc.tensor.matmul(hps[:], lhsT=W1_sb[:], rhs=aT[:], start=True, stop=True)
                hT = ep.tile([128, P], f32, tag="h1T")
                nc.scalar.activation(hT[:], hps[:], ACTF.Relu,
                                     bias=vecs_sb[:, 1:2], scale=vecs_sb[:, 0:1])
                hp = paux.tile([P, 128], f32, tag="mm")
                nc.tensor.transpose(hp[:], hT[:], ident[:])
                hb = ep.tile([P, WTAB], bf16, tag="h1b")
                nc.scalar.activation(hb[:], hp[:], ACTF.Copy, scale=dis_sb[:, t:t + 1])
                wr_cc(0, t, hb[:])

            phase(0, D_IN, ep1)

            # ================= Phase 2: L2 + dense L3 =================
            def ep2(t, ps):
                a = ep.tile([P, 128], f32, tag="a2")
                nc.scalar.activation(a[:], ps[:], ACTF.Copy, scale=dis_sb[:, t:t + 1])
                aT = transpose_f32(a, P, 128)
                y3ps = paux.tile([128, P], f32, tag="acc")
                for h in range(2):
                    hps = paux.tile([128, P], f32, tag="mm")
                    nc.tensor.matmul(hps[:], lhsT=W2_sb[:, h * 128:(h + 1) * 128],
                                     rhs=aT[:], start=True, stop=True)
                    hT = ep.tile([128, P], f32, tag="h2T")
                    nc.scalar.activation(hT[:], hps[:], ACTF.Relu,
                                         bias=vecs_sb[:, 3 + 2 * h:4 + 2 * h],
                                         scale=vecs_sb[:, 2 + 2 * h:3 + 2 * h])
                    nc.tensor.matmul(y3ps[:], lhsT=W3_sb[:, h * 128:(h + 1) * 128],
                                     rhs=hT[:], start=(h == 0), stop=(h == 1))
                y3T = ep.tile([128, P], f32, tag="y3T")
                nc.vector.tensor_copy(out=y3T[:], in_=y3ps[:])
                y3p = paux.tile([P, 128], f32, tag="mm")
                nc.tensor.transpose(y3p[:], y3T[:], ident[:])
                y3b = ep.tile([P, WTAB], bf16, tag="y3b")
                nc.scalar.activation(y3b[:], y3p[:], ACTF.Copy, scale=dis_sb[:, t:t + 1])
                wr_cc(1, t, y3b[:])

            phase(1, 128, ep2)

            # ================= Phase 3: L3 agg + dense L4 =================
            def ep3(t, ps):
                z = ep.tile([P, 128], f32, tag="z3")
                nc.scalar.activation(z[:], ps[:], ACTF.Copy, scale=dis_sb[:, t:t + 1])
                zT = transpose_f32(z, P, 128)
                h3T = ep.tile([128, P], f32, tag="h3T")
                nc.scalar.activation(h3T[:], zT[:], ACTF.Relu,
                                     bias=vecs_sb[:, 7:8], scale=vecs_sb[:, 6:7])
                y4ps = paux.tile([64, P], f32, tag="mm")
                nc.tensor.matmul(y4ps[:], lhsT=W4_sb[:], rhs=h3T[:], start=True, stop=True)
                y4T = ep.tile([64, P], f32, tag="y4T")
                nc.vector.tensor_copy(out=y4T[:], in_=y4ps[:])
                y4p = paux.tile([P, 64], f32, tag="mm")
                nc.tensor.transpose(y4p[:], y4T[:], ident[:64, :64])
                y4b = ep.tile([P, WTAB], bf16, tag="y4b")
                nc.vector.memset(y4b[:, 64:], 0)
                nc.scalar.activation(y4b[:, :64], y4p[:], ACTF.Copy,
                                     scale=dis_sb[:, t:t + 1])
                wr_cc(2, t, y4b[:])

            phase(2, 128, ep3)

            # ================= Phase 4: L4 agg + classifier =================
            def ep4(t, ps):
                z = ep.tile([P, 64], f32, tag="z4")
                nc.scalar.activation(z[:], ps[:], ACTF.Copy, scale=dis_sb[:, t:t + 1])
                zT = transpose_f32(z, P, 64)
                h4T = ep.tile([64, P], f32, tag="h4T")
                nc.scalar.activation(h4T[:], zT[:], ACTF.Relu,
                                     bias=vecs_sb[:64, 9:10], scale=vecs_sb[:64, 8:9])
                u1ps = paux.tile([64, P], f32, tag="mm")
                nc.tensor.matmul(u1ps[:], lhsT=cW1_sb[:], rhs=h4T[:], start=True, stop=True)
                u1T = ep.tile([64, P], f32, tag="u1T")
                nc.scalar.activation(u1T[:], u1ps[:], ACTF.Relu, bias=vecs_sb[:64, 10:11])
                u2ps = paux.tile([32, P], f32, tag="mm")
                nc.tensor.matmul(u2ps[:], lhsT=cW2_sb[:], rhs=u1T[:], start=True, stop=True)
                u2T = ep.tile([32, P], f32, tag="u2T")
                nc.scalar.activation(u2T[:], u2ps[:], ACTF.Relu, bias=vecs_sb[:32, 11:12])
                ops_ = paux.tile([2, P], f32, tag="mm")
                nc.tensor.matmul(ops_[:], lhsT=cW3_sb[:], rhs=u2T[:], start=True, stop=True)
                oT = ep.tile([2, P], f32, tag="oT")
                nc.scalar.activation(oT[:], ops_[:], ACTF.Identity, bias=vecs_sb[:2, 12:13])
                nc.sync.dma_start(out=t_out[:, t * P:(t + 1) * P], in_=oT[:])

            phase(3, 64, ep4)

    nc.finalize()
    return nc


_CACHE = {}


def kernel(**inputs):
    x = np.asarray(inputs["x"], dtype=np.float32)
    edge_index = np.asarray(inputs["edge_index"])
    N = x.shape[0]
    key = hashlib.sha256(edge_index.tobytes()).hexdigest()[:16] + f"_{N}_{x.shape[1]}"
    if key not in _CACHE:
        meta = _prep(x, edge_index, inputs, N)
        nc = _build(meta)
        _CACHE[key] = (meta, nc)
    else:
        meta, nc = _CACHE[key]
        # x may differ between calls with same graph: recompute xt
        meta = dict(meta)
        m2 = _prep(x, edge_index, inputs, N)
        meta["xt"] = m2["xt"]
        meta.update({k: m2[k] for k in ("vecs", "W1", "W2", "W3", "W4", "cW1", "cW2p", "cW3p", "dis_t")})

    rng0, csh = meta["rng0"], meta["csh"]
    in_maps = []
    for d in range(NDEV):
        im = {
            "idx": meta["idx_w"][d],
            "dstl": meta["dstl_w"][d],
            "dis": meta["dis_t"][d],
            "vecs": meta["vecs"],
            "W1": meta["W1"], "W2": meta["W2"], "W3": meta["W3"], "W4": meta["W4"],
            "cW1": meta["cW1"], "cW2p": meta["cW2p"], "cW3p": meta["cW3p"],
        }
        for r in range(NR):
            im[f"xt{r}"] = meta["xt"][rng0[r]:rng0[r + 1]]
        im["xt_own"] = np.concatenate([
            meta["xt"][rng0[c] + d * csh[c]:rng0[c] + (d + 1) * csh[c]]
            for c in range(NR)
        ])
        in_maps.append(im)
    res = None
    for _attempt in range(4):
        try:
            res = run_bass_kernel_spmd(nc, in_maps, core_ids=list(range(NDEV)), trace=False)
            break
        except Exception:
            if _attempt == 3:
                raise

    SHARD = meta["SHARD"]
    out = np.empty((N, 2), dtype=np.float32)
    for d in range(NDEV):
        out[d * SHARD:(d + 1) * SHARD] = res.results[d]["outT"][:, :SHARD].T
    return out


# revision 7
# speedup vs baseline: 1.0344x; 1.0344x over previous
"""Trainium2 Bass kernel for nn_EnhancedGCN42 (4-layer GCN + MLP classifier).

Strategy (8 NeuronCores, SPMD single NEFF):
  - Nodes dst-sharded: device d owns dst nodes [d*12500, (d+1)*12500).
  - A-hat = D^-1/2 (A+I) D^-1/2 factorized: tables store dis*h rows (bf16,
    256B rows); aggregation output scaled by dis_dst.
  - Row space is chunk-major with uneven chunks (16,16,16,16,16,16,2 tiles
    per device). Each phase's table lives in 7 separate DRAM tensors (one
    per chunk) so the AllGather of chunk c and the gathers that read it
    have exact deps: chunk c's AllGather is issued on the sync engine
    right after its block's epilogue writes (so it never blocks the
    gpsimd gather-dispatch stream), and next-phase gathers for range r
    wait only on chunk r. The tiny last chunk minimizes the refill
    bubble at phase boundaries.
  - Per layer: per-edge rows gathered via dma_gather (4 SWDGE queues),
    aggregated per 128-dst tile by matmul with an on-chip-built one-hot
    selection matrix (per-use vector tensor_scalar is_equal, 4x DVE).
  - Dense W / BN / ReLU fused per dst-tile in transposed layout; BN and
    classifier BN folded on host into per-feature scale/bias.

kernel(**inputs) -> [100000, 2] float32.
"""
import hashlib
import numpy as np
import ml_dtypes

import concourse.bacc as bacc
import concourse.bass as bass
import concourse.mybir as mybir
import concourse.tile as tile
from concourse.masks import make_identity
from concourse.bass_utils import run_bass_kernel_spmd

f32 = mybir.dt.float32
bf16 = mybir.dt.bfloat16
i16 = mybir.dt.int16
i32 = mybir.dt.int32
nbf16 = ml_dtypes.bfloat16

P = 128
NDEV = 8
EPS = 1e-5
WTAB = 128       # table row = 128 cols bf16 = 256B
CH_TILES = [16, 16, 16, 16, 16, 14, 4]   # tiles per chunk (sum = NT = 98)
NR = len(CH_TILES)


def _prep(x, edge_index, params, N):
    """Host preprocessing: graph partition + folded constants. Returns meta dict."""
    SHARD = N // NDEV                       # 12500
    TSHARD = ((SHARD + P - 1) // P) * P     # 12544
    NT = TSHARD // P                        # 98
    TROWS = TSHARD * NDEV                   # 100352
    assert sum(CH_TILES) == NT
    ch_t0 = np.cumsum([0] + CH_TILES)                    # tile offsets [NR+1]
    csh = [ct * P for ct in CH_TILES]                    # rows/device/chunk
    rngw = [c * NDEV for c in csh]                       # rows/chunk (global)
    rng0 = np.cumsum([0] + rngw)                         # chunk row offsets
    assert all(w <= 32768 for w in rngw)

    ei = edge_index.astype(np.int64)
    loop = np.arange(N, dtype=np.int64)
    dst_all = np.concatenate([ei[1], loop])
    deg = np.bincount(dst_all, minlength=N).astype(np.float32)
    dis = (1.0 / np.sqrt(deg)).astype(np.float32)

    def padrow(n):
        # node -> chunk-major padded row. local tile lt = local//128 belongs
        # to chunk c(lt); row = rng0[c] + d*csh[c] + (local - ch_t0[c]*128)
        d = n // SHARD
        local = n - d * SHARD
        lt = local // P
        c = np.searchsorted(ch_t0, lt // 1 + 1) - 1  # chunk of tile lt
        c = np.minimum(c, NR - 1)
        return rng0[c] + d * np.array(csh)[c] + (local - ch_t0[c] * P)

    src_e = ei[0]
    dst_e = ei[1]
    psrc_e = padrow(src_e)
    r_of_row = lambda rows: np.searchsorted(rng0, rows, side="right") - 1

    counts = np.zeros((NDEV, NT, NR), dtype=np.int64)
    dev_edges = []
    for d in range(NDEV):
        m = (dst_e >= d * SHARD) & (dst_e < (d + 1) * SHARD)
        es = psrc_e[m]
        el = dst_e[m] - d * SHARD
        t_id = el >> 7
        r_id = r_of_row(es)
        order = np.lexsort((es, r_id, t_id))  # (tile, range, src-ascending)
        es, el, t_id, r_id = es[order], el[order], t_id[order], r_id[order]
        np.add.at(counts[d], (t_id, r_id), 1)
        dev_edges.append((es, el, r_id))

    grp_rows = counts.max(axis=0).astype(np.int64)  # [NT, NR] exact max

    n_blk = NR  # gather blocks == chunks (uneven tile counts)
    grp_off = np.zeros((NT, NR), dtype=np.int64)
    blk_off = np.zeros((n_blk, NR), dtype=np.int64)
    blk_rows = np.zeros((n_blk, NR), dtype=np.int64)
    acc = 0
    for b in range(n_blk):
        for r in range(NR):
            blk_off[b, r] = acc
            for t in range(ch_t0[b], ch_t0[b + 1]):
                grp_off[t, r] = acc
                acc += grp_rows[t, r]
            acc = (acc + P - 1) // P * P  # pad gather to whole chunks
            blk_rows[b, r] = acc - blk_off[b, r]
    TOT = acc
    assert TOT % 16 == 0

    # chunk-use enumeration: per (t, r) the 128-row chunks its group overlaps.
    uses = [[[] for _ in range(NR)] for _ in range(NT)]  # (local_col, dstl_col)
    n_uses = 0
    for b in range(n_blk):
        for r in range(NR):
            for t in range(ch_t0[b], ch_t0[b + 1]):
                g0, g1 = grp_off[t, r], grp_off[t, r] + grp_rows[t, r]
                if g1 == g0:
                    continue
                c0, c1 = int(g0 // P), int((g1 + P - 1) // P)
                for ci in range(c0, c1):
                    uses[t][r].append((ci - int(blk_off[b, r]) // P, n_uses, ci))
                    n_uses += 1

    idx_w = np.zeros((NDEV, P, TOT // 16), dtype=np.int16)
    dstl_w = np.full((NDEV, P, n_uses), 255.0, dtype=np.float32)
    for d in range(NDEV):
        es, el, _r = dev_edges[d]
        IDX = np.zeros(TOT, dtype=np.int16)
        DLOC = np.full(TOT, 255.0, dtype=np.float32)
        OWNER = np.full(TOT, -1, dtype=np.int64)
        pos = 0
        for t in range(NT):
            for r in range(NR):
                c = int(counts[d, t, r])
                o = int(grp_off[t, r])
                IDX[o:o + c] = (es[pos:pos + c] - rng0[r]).astype(np.int16)
                DLOC[o:o + c] = (el[pos:pos + c] - t * P).astype(np.float32)
                OWNER[o:o + int(grp_rows[t, r])] = t
                pos += c
        idx_w[d] = np.tile(IDX.reshape(-1, 16).T, (8, 1))
        D = dstl_w[d]
        for t in range(NT):
            for r in range(NR):
                for (_lc, du, ci) in uses[t][r]:
                    rows = np.arange(ci * P, (ci + 1) * P)
                    v = np.where(OWNER[rows] == t, DLOC[rows], 255.0)
                    D[:, du] = v

    dis_pad = np.zeros(TROWS, dtype=np.float32)
    dis_pad[padrow(np.arange(N))] = dis
    dis_t = np.zeros((NDEV, P, NT), dtype=np.float32)
    for d in range(NDEV):
        for t in range(NT):
            c = int(np.searchsorted(ch_t0, t + 1) - 1)
            g0 = rng0[c] + d * csh[c] + (t - ch_t0[c]) * P
            dis_t[d, :, t] = dis_pad[g0:g0 + P]

    # x-tilde table (bf16, padded cols), chunk-major row space
    xt = np.zeros((TROWS, WTAB), dtype=nbf16)
    v = (dis[:, None] * x).astype(nbf16)
    xt[padrow(np.arange(N)), :x.shape[1]] = v

    def fold(g, be, rm, rv, b):
        k = (1.0 / np.sqrt(rv + EPS)).astype(np.float32)
        s = g * k
        t = (b - rm) * s + be
        return s.astype(np.float32), t.astype(np.float32)

    s1, t1 = fold(params["g1"], params["be1"], params["rm1"], params["rv1"], params["b1"])
    s2, t2 = fold(params["g2"], params["be2"], params["rm2"], params["rv2"], params["b2"])
    s3, t3 = fold(params["g3"], params["be3"], params["rm3"], params["rv3"], params["b3"])
    s4, t4 = fold(params["g4"], params["be4"], params["rm4"], params["rv4"], params["b4"])
    zk = (1.0 / np.sqrt(params["crv1"] + EPS)).astype(np.float32)
    cs1 = params["cg1"] * zk
    ct1 = -params["crm1"] * cs1 + params["cbe1"]
    zk = (1.0 / np.sqrt(params["crv2"] + EPS)).astype(np.float32)
    cs2 = params["cg2"] * zk
    ct2 = -params["crm2"] * cs2 + params["cbe2"]
    cW2p = (cs1[:, None] * params["cW2"]).astype(np.float32)
    cb2p = (ct1 @ params["cW2"] + params["cb2"]).astype(np.float32)
    cW3p = (cs2[:, None] * params["cW3"]).astype(np.float32)
    cb3p = (ct2 @ params["cW3"] + params["cb3"]).astype(np.float32)

    vecs = np.zeros((P, 13), dtype=np.float32)
    vecs[:, 0], vecs[:, 1] = s1, t1
    vecs[:, 2], vecs[:, 3] = s2[:128], t2[:128]
    vecs[:, 4], vecs[:, 5] = s2[128:], t2[128:]
    vecs[:, 6], vecs[:, 7] = s3, t3
    vecs[:64, 8], vecs[:64, 9] = s4, t4
    vecs[:64, 10] = params["cb1"]
    vecs[:32, 11] = cb2p
    vecs[:2, 12] = cb3p

    return dict(
        N=N, SHARD=SHARD, TSHARD=TSHARD, NT=NT, TROWS=TROWS,
        ch_t0=ch_t0, csh=csh, rngw=rngw, rng0=rng0,
        TOT=TOT, uses=uses, n_uses=n_uses,
        n_blk=n_blk, blk_off=blk_off, blk_rows=blk_rows,
        idx_w=idx_w, dstl_w=dstl_w, dis_t=dis_t, xt=xt, vecs=vecs,
        W1=params["W1"].astype(np.float32), W2=params["W2"].astype(np.float32),
        W3=np.concatenate([params["W3"][:128], params["W3"][128:]], axis=1).astype(np.float32),
        W4=params["W4"].astype(np.float32),
        cW1=params["cW1"].astype(np.float32), cW2p=cW2p, cW3p=cW3p,
        d_in=x.shape[1],
    )


def _build(meta):
    """Build the Bass program (same for all cores)."""
    NT, TSHARD, TROWS = meta["NT"], meta["TSHARD"], meta["TROWS"]
    ch_t0, csh, rngw = meta["ch_t0"], meta["csh"], meta["rngw"]
    TOT = meta["TOT"]
    uses, n_uses = meta["uses"], meta["n_uses"]
    n_blk, blk_off, blk_rows = meta["n_blk"], meta["blk_off"], meta["blk_rows"]
    D_IN = meta["d_in"]

    nc = bacc.Bacc(None, target_bir_lowering=False, num_swdge_queues=4)
    t_xt = [nc.dram_tensor(f"xt{r}", [rngw[r], WTAB], bf16, kind="ExternalInput")
            for r in range(NR)]
    t_idx = nc.dram_tensor("idx", [P, TOT // 16], i16, kind="ExternalInput")
    t_dstl = nc.dram_tensor("dstl", [P, n_uses], f32, kind="ExternalInput")
    t_xto = nc.dram_tensor("xt_own", [TSHARD, WTAB], bf16, kind="ExternalInput")
    t_dis = nc.dram_tensor("dis", [P, NT], f32, kind="ExternalInput")
    t_vecs = nc.dram_tensor("vecs", [P, 13], f32, kind="ExternalInput")
    t_W1 = nc.dram_tensor("W1", [D_IN, 128], f32, kind="ExternalInput")
    t_W2 = nc.dram_tensor("W2", [128, 256], f32, kind="ExternalInput")
    t_W3 = nc.dram_tensor("W3", [128, 256], f32, kind="ExternalInput")  # packed K-halves
    t_W4 = nc.dram_tensor("W4", [128, 64], f32, kind="ExternalInput")
    t_cW1 = nc.dram_tensor("cW1", [64, 64], f32, kind="ExternalInput")
    t_cW2 = nc.dram_tensor("cW2p", [64, 32], f32, kind="ExternalInput")
    t_cW3 = nc.dram_tensor("cW3p", [32, 2], f32, kind="ExternalInput")
    t_out = nc.dram_tensor("outT", [2, TSHARD], f32, kind="ExternalOutput")

    cc_in = [[nc.dram_tensor(f"cc_in{k}_{c}", [csh[c], WTAB], bf16) for c in range(NR)]
             for k in range(3)]
    tabs = [[nc.dram_tensor(f"tab{k}_{c}", [rngw[c], WTAB], bf16, addr_space="Shared")
             for c in range(NR)] for k in range(3)]

    qctr = [0]

    def qrr():
        qctr[0] = (qctr[0] + 1) % 4
        return qctr[0]

    with tile.TileContext(nc) as tc:
        with (
            tc.tile_pool(name="const", bufs=1) as cpool,
            tc.tile_pool(name="gp", bufs=8) as gpool,
            tc.tile_pool(name="sp", bufs=8) as spool,
            tc.tile_pool(name="pagg", bufs=2, space="PSUM") as pagg,
            tc.tile_pool(name="paux", bufs=3, space="PSUM") as paux,
            tc.tile_pool(name="ep", bufs=3) as ep,
        ):
            # ---- constants
            idx_sb = cpool.tile([P, TOT // 16], i16)
            nc.sync.dma_start(out=idx_sb[:], in_=t_idx[:])
            dstlf_sb = cpool.tile([P, n_uses], f32)
            nc.sync.dma_start(out=dstlf_sb[:], in_=t_dstl[:])
            dstl_sb = cpool.tile([P, n_uses], bf16)
            nc.vector.tensor_copy(out=dstl_sb[:], in_=dstlf_sb[:])
            dstln_sb = cpool.tile([P, n_uses], f32)
            nc.vector.tensor_scalar_mul(dstln_sb[:], dstlf_sb[:], -1.0)
            dis_sb = cpool.tile([P, NT], f32)
            nc.sync.dma_start(out=dis_sb[:], in_=t_dis[:])
            vecs_sb = cpool.tile([P, 13], f32)
            nc.sync.dma_start(out=vecs_sb[:], in_=t_vecs[:])
            W1_sb = cpool.tile([D_IN, 128], f32)
            nc.sync.dma_start(out=W1_sb[:], in_=t_W1[:])
            W2_sb = cpool.tile([128, 256], f32)
            nc.sync.dma_start(out=W2_sb[:], in_=t_W2[:])
            W3_sb = cpool.tile([128, 256], f32)
            nc.sync.dma_start(out=W3_sb[:], in_=t_W3[:])
            W4_sb = cpool.tile([128, 64], f32)
            nc.sync.dma_start(out=W4_sb[:], in_=t_W4[:])
            cW1_sb = cpool.tile([64, 64], f32)
            nc.sync.dma_start(out=cW1_sb[:], in_=t_cW1[:])
            cW2_sb = cpool.tile([64, 32], f32)
            nc.sync.dma_start(out=cW2_sb[:], in_=t_cW2[:])
            cW3_sb = cpool.tile([32, 2], f32)
            nc.sync.dma_start(out=cW3_sb[:], in_=t_cW3[:])
            ident = cpool.tile([P, P], f32)
            make_identity(nc, ident[:])
            ident_bf = cpool.tile([P, P], bf16)
            nc.vector.tensor_copy(out=ident_bf[:], in_=ident[:])
            KMAX = max((len(uses[t][r]) for t in range(NT) for r in range(NR)),
                       default=1)
            iota_i = cpool.tile([P, KMAX, P], i32)
            nc.gpsimd.iota(iota_i[:], pattern=[[0, KMAX], [1, P]], base=0,
                           channel_multiplier=0)
            iota_bf = cpool.tile([P, KMAX, P], bf16)
            nc.vector.tensor_copy(out=iota_bf[:], in_=iota_i[:])

            AluEq = mybir.AluOpType.is_equal
            ACTF = mybir.ActivationFunctionType

            def transpose_f32(src_sb, pdim, fdim):
                """[pdim, fdim] f32 sbuf -> [fdim, pdim] f32 sbuf (PE transpose)."""
                tp = paux.tile([fdim, pdim], f32, tag="mm")
                nc.tensor.transpose(tp[:], src_sb[:], ident[:pdim, :pdim])
                out = ep.tile([fdim, pdim], f32, tag="tps")
                nc.vector.tensor_copy(out=out[:], in_=tp[:])
                return out

            def emit_ag(k, c):
                # chunk-c table of phase k for the next phase. Emitted with a
                # ~2-block lag so its input-ready wait is already satisfied
                # when the gpsimd dispatcher reaches it (no head-of-line
                # block of the gather stream).
                nc.gpsimd.collective_compute(
                    "AllGather", mybir.AluOpType.bypass,
                    replica_groups=[list(range(NDEV))],
                    ins=[cc_in[k][c][:]], outs=[tabs[k][c][:]],
                )

            def phase(k, w, epilogue):
                """Block-merged gathers + per-tile S-matmul aggregation."""
                if k > 0:
                    emit_ag(k - 1, 5)
                for b in range(n_blk):
                    tiles = range(ch_t0[b], ch_t0[b + 1])
                    gt = {}
                    for r in range(NR):
                        if k > 0 and b == 0 and r == 5:
                            emit_ag(k - 1, 6)
                        rows = int(blk_rows[b, r])
                        if rows == 0:
                            continue
                        g = gpool.tile([P, rows // P, WTAB], bf16, tag="g")
                        off = int(blk_off[b, r])
                        table_ap = (t_xt[r] if k == 0 else tabs[k - 1][r])
                        nc.gpsimd.dma_gather(
                            out_ap=g[:],
                            in_ap=table_ap[:, :],
                            idxs_ap=idx_sb[:, off // 16:(off + rows) // 16],
                            num_idxs=rows,
                            num_idxs_reg=rows,
                            elem_size=WTAB,
                            single_packet=False,
                            queue_num=qrr(),
                        )
                        gt[r] = g
                    if k < 3 and 0 <= b - 2 <= 4:
                        emit_ag(k, b - 2)
                    for t in tiles:
                        own = ep.tile([P, WTAB], bf16, tag="own")
                        if k == 0:
                            src_ap = t_xto[t * P:(t + 1) * P, :]
                        else:
                            c = int(np.searchsorted(ch_t0, t + 1) - 1)
                            pt = t - ch_t0[c]
                            src_ap = cc_in[k - 1][c][pt * P:(pt + 1) * P, :]
                        nc.sync.dma_start(out=own[:], in_=src_ap)
                        spt = {}
                        for r in range(NR):
                            ul = uses[t][r]
                            if not ul:
                                continue
                            du0 = ul[0][1]
                            sP = spool.tile([P, len(ul), P], bf16, tag="s")
                            if (t * NR + r) % 8 != 7:
                                nc.vector.tensor_tensor(
                                    out=sP[:],
                                    in0=dstl_sb[:, du0:du0 + len(ul)].to_broadcast([P, len(ul), P]),
                                    in1=iota_bf[:, :len(ul), :],
                                    op=AluEq,
                                )
                            else:
                                # ACT path: s = Relu(1 - (iota - dstl)^2)
                                yq = spool.tile([P, len(ul), P], bf16, tag="yq")
                                for ui in range(len(ul)):
                                    nc.scalar.activation(
                                        yq[:, ui, :], iota_bf[:, ui, :], ACTF.Square,
                                        bias=dstln_sb[:, du0 + ui:du0 + ui + 1])
                                nc.scalar.activation(sP[:], yq[:], ACTF.Relu,
                                                     bias=1.0, scale=-1.0)
                            spt[r] = sP
                        nmm = 1 + sum(len(uses[t][r]) for r in range(NR))
                        ps = pagg.tile([P, w], f32, tag="pagg")
                        nc.tensor.matmul(ps[:], lhsT=ident_bf[:], rhs=own[:, :w],
                                         start=True, stop=(nmm == 1))
                        kk = 1
                        for r in range(NR):
                            for ui, (lc, du, _ci) in enumerate(uses[t][r]):
                                nc.tensor.matmul(
                                    ps[:], lhsT=spt[r][:, ui, :], rhs=gt[r][:, lc, :w],
                                    start=False, stop=(kk == nmm - 1),
                                )
                                kk += 1
                        epilogue(t, ps)

            def wr_cc(k, t, src):
                c = int(np.searchsorted(ch_t0, t + 1) - 1)
                pt = t - ch_t0[c]
                nc.sync.dma_start(out=cc_in[k][c][pt * P:(pt + 1) * P, :], in_=src)

            # ================= Phase 1: L1 =================
            def ep1(t, ps):
                a = ep.tile([P, D_IN], f32, tag="a1")
                nc.scalar.activation(a[:], ps[:], ACTF.Copy, scale=dis_sb[:, t:t + 1])
                aT = transpose_f32(a, P, D_IN)
                hps = paux.tile([128, P], f32, tag="mm")
                nc.tensor.matmul(hps[:], lhsT=W1_sb[:], rhs=aT[:], start=True, stop=True)
                hT = ep.tile([128, P], f32, tag="h1T")
                nc.scalar.activation(hT[:], hps[:], ACTF.Relu,
                                     bias=vecs_sb[:, 1:2], scale=vecs_sb[:, 0:1])
                hp = paux.tile([P, 128], f32, tag="mm")
                nc.tensor.transpose(hp[:], hT[:], ident[:])
                hb = ep.tile([P, WTAB], bf16, tag="h1b")
                nc.scalar.activation(hb[:], hp[:], ACTF.Copy, scale=dis_sb[:, t:t + 1])
                wr_cc(0, t, hb[:])

            phase(0, D_IN, ep1)

            # ================= Phase 2: L2 + dense L3 =================
            def ep2(t, ps):
                a = ep.tile([P, 128], f32, tag="a2")
                nc.scalar.activation(a[:], ps[:], ACTF.Copy, scale=dis_sb[:, t:t + 1])
                aT = transpose_f32(a, P, 128)
                y3ps = paux.tile([128, P], f32, tag="acc")
                for h in range(2):
                    hps = paux.tile([128, P], f32, tag="mm")
                    nc.tensor.matmul(hps[:], lhsT=W2_sb[:, h * 128:(h + 1) * 128],
                                     rhs=aT[:], start=True, stop=True)
                    hT = ep.tile([128, P], f32, tag="h2T")
                    nc.scalar.activation(hT[:], hps[:], ACTF.Relu,
                                         bias=vecs_sb[:, 3 + 2 * h:4 + 2 * h],
                                         scale=vecs_sb[:, 2 + 2 * h:3 + 2 * h])
                    nc.tensor.matmul(y3ps[:], lhsT=W3_sb[:, h * 128:(h + 1) * 128],
                                     rhs=hT[:], start=(h == 0), stop=(h == 1))
                y3T = ep.tile([128, P], f32, tag="y3T")
                nc.vector.tensor_copy(out=y3T[:], in_=y3ps[:])
                y3p = paux.tile([P, 128], f32, tag="mm")
                nc.tensor.transpose(y3p[:], y3T[:], ident[:])
                y3b = ep.tile([P, WTAB], bf16, tag="y3b")
                nc.scalar.activation(y3b[:], y3p[:], ACTF.Copy, scale=dis_sb[:, t:t + 1])
                wr_cc(1, t, y3b[:])

            phase(1, 128, ep2)

            # ================= Phase 3: L3 agg + dense L4 =================
            def ep3(t, ps):
                z = ep.tile([P, 128], f32, tag="z3")
                nc.scalar.activation(z[:], ps[:], ACTF.Copy, scale=dis_sb[:, t:t + 1])
                zT = transpose_f32(z, P, 128)
                h3T = ep.tile([128, P], f32, tag="h3T")
                nc.scalar.activation(h3T[:], zT[:], ACTF.Relu,
                                     bias=vecs_sb[:, 7:8], scale=vecs_sb[:, 6:7])
                y4ps = paux.tile([64, P], f32, tag="mm")
                nc.tensor.matmul(y4ps[:], lhsT=W4_sb[:], rhs=h3T[:], start=True, stop=True)
                y4T = ep.tile([64, P], f32, tag="y4T")
                nc.vector.tensor_copy(out=y4T[:], in_=y4ps[:])
                y4p = paux.tile([P, 64], f32, tag="mm")
                nc.tensor.transpose(y4p[:], y4T[:], ident[:64, :64])
                y4b = ep.tile([P, WTAB], bf16, tag="y4b")
                nc.vector.memset(y4b[:, 64:], 0)
                nc.scalar.activation(y4b[:, :64], y4p[:], ACTF.Copy,
                                     scale=dis_sb[:, t:t + 1])
                wr_cc(2, t, y4b[:])

            phase(2, 128, ep3)

            # ================= Phase 4: L4 agg + classifier =================
            def ep4(t, ps):
                z = ep.tile([P, 64], f32, tag="z4")
                nc.scalar.activation(z[:], ps[:], ACTF.Copy, scale=dis_sb[:, t:t + 1])
                zT = transpose_f32(z, P, 64)
                h4T = ep.tile([64, P], f32, tag="h4T")
                nc.scalar.activation(h4T[:], zT[:], ACTF.Relu,
                                     bias=vecs_sb[:64, 9:10], scale=vecs_sb[:64, 8:9])
                u1ps = paux.tile([64, P], f32, tag="mm")
                nc.tensor.matmul(u1ps[:], lhsT=cW1_sb[:], rhs=h4T[:], start=True, stop=True)
                u1T = ep.tile([64, P], f32, tag="u1T")
                nc.scalar.activation(u1T[:], u1ps[:], ACTF.Relu, bias=vecs_sb[:64, 10:11])
                u2ps = paux.tile([32, P], f32, tag="mm")
                nc.tensor.matmul(u2ps[:], lhsT=cW2_sb[:], rhs=u1T[:], start=True, stop=True)
                u2T = ep.tile([32, P], f32, tag="u2T")
                nc.scalar.activation(u2T[:], u2ps[:], ACTF.Relu, bias=vecs_sb[:32, 11:12])
                ops_ = paux.tile([2, P], f32, tag="mm")
                nc.tensor.matmul(ops_[:], lhsT=cW3_sb[:], rhs=u2T[:], start=True, stop=True)
                oT = ep.tile([2, P], f32, tag="oT")
                nc.scalar.activation(oT[:], ops_[:], ACTF.Identity, bias=vecs_sb[:2, 12:13])
                nc.sync.dma_start(out=t_out[:, t * P:(t + 1) * P], in_=oT[:])

            phase(3, 64, ep4)

    nc.finalize()
    return nc


_CACHE = {}


def kernel(**inputs):
    x = np.asarray(inputs["x"], dtype=np.float32)
    edge_index = np.asarray(inputs["edge_index"])
    N = x.shape[0]
    key = hashlib.sha256(edge_index.tobytes()).hexdigest()[:16] + f"_{N}_{x.shape[1]}"
    if key not in _CACHE:
        meta = _prep(x, edge_index, inputs, N)
        nc = _build(meta)
        _CACHE[key] = (meta, nc)
    else:
        meta, nc = _CACHE[key]
        # x may differ between calls with same graph: recompute xt
        meta = dict(meta)
        m2 = _prep(x, edge_index, inputs, N)
        meta["xt"] = m2["xt"]
        meta.update({k: m2[k] for k in ("vecs", "W1", "W2", "W3", "W4", "cW1", "cW2p", "cW3p", "dis_t")})

    rng0, csh = meta["rng0"], meta["csh"]
    in_maps = []
    for d in range(NDEV):
        im = {
            "idx": meta["idx_w"][d],
            "dstl": meta["dstl_w"][d],
            "dis": meta["dis_t"][d],
            "vecs": meta["vecs"],
            "W1": meta["W1"], "W2": meta["W2"], "W3": meta["W3"], "W4": meta["W4"],
            "cW1": meta["cW1"], "cW2p": meta["cW2p"], "cW3p": meta["cW3p"],
        }
        for r in range(NR):
            im[f"xt{r}"] = meta["xt"][rng0[r]:rng0[r + 1]]
        im["xt_own"] = np.concatenate([
            meta["xt"][rng0[c] + d * csh[c]:rng0[c] + (d + 1) * csh[c]]
            for c in range(NR)
        ])
        in_maps.append(im)
    res = None
    for _attempt in range(4):
        try:
            res = run_bass_kernel_spmd(nc, in_maps, core_ids=list(range(NDEV)), trace=False)
            break
        except Exception:
            if _attempt == 3:
                raise

    SHARD = meta["SHARD"]
    out = np.empty((N, 2), dtype=np.float32)
    for d in range(NDEV):
        out[d * SHARD:(d + 1) * SHARD] = res.results[d]["outT"][:, :SHARD].T
    return out


# revision 9
# speedup vs baseline: 1.8569x; 1.7950x over previous
"""Trainium2 Bass kernel for nn_EnhancedGCN42 (4-layer GCN + MLP classifier).

Strategy (8 NeuronCores, SPMD single NEFF):
  - Nodes dst-sharded: device d owns dst nodes [d*12500, (d+1)*12500).
  - A-hat = D^-1/2 (A+I) D^-1/2 factorized: tables store dis*h rows (bf16,
    256B rows); aggregation output scaled by dis_dst.
  - Row space is chunk-major with 2 chunks of 49 tiles per device. Each
    phase's table lives in 2 DRAM tensors (one per chunk) so collective ->
    gather deps are exact per chunk. AllGathers have a large (~100us)
    fixed cost, so only 2 per phase; their latency is hidden by emission
    scheduling: chunk-0's AG fires mid-phase (once its 49 tiles are done),
    chunk-1's AG fires at the start of the next phase, and the next
    phase's gathers are ordered so chunk-1-range gathers of the first 4
    blocks are deferred (catch-up at iterations 4..7) until that AG has
    landed. The gpsimd stream (gather descriptor generation, the
    critical resource) then never head-of-line blocks on collectives.
  - Per layer: per-edge rows gathered via dma_gather (4 SWDGE queues),
    aggregated per 128-dst tile by matmul with an on-chip-built one-hot
    selection matrix (is_equal against iota on vector, 1/8 offloaded to
    scalar via a Relu(1-(iota-dstl)^2) trick).
  - Dense W / BN / ReLU fused per dst-tile in transposed layout; BN and
    classifier BN folded on host into per-feature scale/bias.

kernel(**inputs) -> [100000, 2] float32.
"""
import hashlib
import numpy as np
import ml_dtypes

import concourse.bacc as bacc
import concourse.bass as bass
import concourse.mybir as mybir
import concourse.tile as tile
from concourse.masks import make_identity
from concourse.bass_utils import run_bass_kernel_spmd

f32 = mybir.dt.float32
bf16 = mybir.dt.bfloat16
i16 = mybir.dt.int16
i32 = mybir.dt.int32
nbf16 = ml_dtypes.bfloat16

P = 128
NDEV = 8
EPS = 1e-5
WTAB = 128       # table row = 128 cols bf16 = 256B
NCH = 2          # allgather chunks per phase
CHT = 49         # tiles per chunk
NR = 4           # gather src ranges (2 per chunk; int16 limit 25088<=32768)
BLKT = 7         # tiles per gather block
NBLK = 14        # blocks per phase
DEFER = 4        # blocks whose chunk-1-range gathers are deferred


def _prep(x, edge_index, params, N):
    """Host preprocessing: graph partition + folded constants. Returns meta dict."""
    SHARD = N // NDEV                       # 12500
    TSHARD = ((SHARD + P - 1) // P) * P     # 12544
    NT = TSHARD // P                        # 98
    TROWS = TSHARD * NDEV                   # 100352
    CSH = CHT * P                           # 6272 rows per device per chunk
    RNGW = TROWS // NR                      # 25088 rows per range
    assert NT == NCH * CHT == NBLK * BLKT and RNGW <= 32768

    ei = edge_index.astype(np.int64)
    loop = np.arange(N, dtype=np.int64)
    dst_all = np.concatenate([ei[1], loop])
    deg = np.bincount(dst_all, minlength=N).astype(np.float32)
    dis = (1.0 / np.sqrt(deg)).astype(np.float32)

    def padrow(n):
        # node -> chunk-major padded row: chunk c spans all devices' c-th
        # 49-tile sub-shard. row = c*CSH*NDEV + d*CSH + (local - c*CSH)
        d = n // SHARD
        local = n - d * SHARD
        c = local // CSH
        return c * CSH * NDEV + d * CSH + (local - c * CSH)

    src_e = ei[0]
    dst_e = ei[1]
    psrc_e = padrow(src_e)

    counts = np.zeros((NDEV, NT, NR), dtype=np.int64)
    dev_edges = []
    for d in range(NDEV):
        m = (dst_e >= d * SHARD) & (dst_e < (d + 1) * SHARD)
        es = psrc_e[m]
        el = dst_e[m] - d * SHARD
        t_id = el >> 7
        r_id = es // RNGW
        order = np.lexsort((es, r_id, t_id))  # (tile, range, src-ascending)
        es, el, t_id, r_id = es[order], el[order], t_id[order], r_id[order]
        np.add.at(counts[d], (t_id, r_id), 1)
        dev_edges.append((es, el))

    grp_rows = counts.max(axis=0).astype(np.int64)  # [NT, NR] exact max

    grp_off = np.zeros((NT, NR), dtype=np.int64)
    blk_off = np.zeros((NBLK, NR), dtype=np.int64)
    blk_rows = np.zeros((NBLK, NR), dtype=np.int64)
    acc = 0
    for b in range(NBLK):
        for r in range(NR):
            blk_off[b, r] = acc
            for t in range(b * BLKT, (b + 1) * BLKT):
                grp_off[t, r] = acc
                acc += grp_rows[t, r]
            acc = (acc + P - 1) // P * P  # pad gather to whole chunks
            blk_rows[b, r] = acc - blk_off[b, r]
    TOT = acc
    assert TOT % 16 == 0

    # chunk-use enumeration: per (t, r) the 128-row chunks its group overlaps.
    uses = [[[] for _ in range(NR)] for _ in range(NT)]  # (local_col, dstl_col)
    n_uses = 0
    for b in range(NBLK):
        for r in range(NR):
            for t in range(b * BLKT, (b + 1) * BLKT):
                g0, g1 = grp_off[t, r], grp_off[t, r] + grp_rows[t, r]
                if g1 == g0:
                    continue
                c0, c1 = int(g0 // P), int((g1 + P - 1) // P)
                for ci in range(c0, c1):
                    uses[t][r].append((ci - int(blk_off[b, r]) // P, n_uses, ci))
                    n_uses += 1

    idx_w = np.zeros((NDEV, P, TOT // 16), dtype=np.int16)
    dstl_w = np.full((NDEV, P, n_uses), 255.0, dtype=np.float32)
    for d in range(NDEV):
        es, el = dev_edges[d]
        IDX = np.zeros(TOT, dtype=np.int16)
        DLOC = np.full(TOT, 255.0, dtype=np.float32)
        OWNER = np.full(TOT, -1, dtype=np.int64)
        pos = 0
        for t in range(NT):
            for r in range(NR):
                c = int(counts[d, t, r])
                o = int(grp_off[t, r])
                IDX[o:o + c] = (es[pos:pos + c] - r * RNGW).astype(np.int16)
                DLOC[o:o + c] = (el[pos:pos + c] - t * P).astype(np.float32)
                OWNER[o:o + int(grp_rows[t, r])] = t
                pos += c
        idx_w[d] = np.tile(IDX.reshape(-1, 16).T, (8, 1))
        D = dstl_w[d]
        for t in range(NT):
            for r in range(NR):
                for (_lc, du, ci) in uses[t][r]:
                    rows = np.arange(ci * P, (ci + 1) * P)
                    v = np.where(OWNER[rows] == t, DLOC[rows], 255.0)
                    D[:, du] = v

    dis_pad = np.zeros(TROWS, dtype=np.float32)
    dis_pad[padrow(np.arange(N))] = dis
    dis_t = np.zeros((NDEV, P, NT), dtype=np.float32)
    for d in range(NDEV):
        for t in range(NT):
            c = t // CHT
            g0 = c * CSH * NDEV + d * CSH + (t - c * CHT) * P
            dis_t[d, :, t] = dis_pad[g0:g0 + P]

    # x-tilde table (bf16, padded cols), chunk-major row space
    xt = np.zeros((TROWS, WTAB), dtype=nbf16)
    v = (dis[:, None] * x).astype(nbf16)
    xt[padrow(np.arange(N)), :x.shape[1]] = v

    def fold(g, be, rm, rv, b):
        k = (1.0 / np.sqrt(rv + EPS)).astype(np.float32)
        s = g * k
        t = (b - rm) * s + be
        return s.astype(np.float32), t.astype(np.float32)

    s1, t1 = fold(params["g1"], params["be1"], params["rm1"], params["rv1"], params["b1"])
    s2, t2 = fold(params["g2"], params["be2"], params["rm2"], params["rv2"], params["b2"])
    s3, t3 = fold(params["g3"], params["be3"], params["rm3"], params["rv3"], params["b3"])
    s4, t4 = fold(params["g4"], params["be4"], params["rm4"], params["rv4"], params["b4"])
    zk = (1.0 / np.sqrt(params["crv1"] + EPS)).astype(np.float32)
    cs1 = params["cg1"] * zk
    ct1 = -params["crm1"] * cs1 + params["cbe1"]
    zk = (1.0 / np.sqrt(params["crv2"] + EPS)).astype(np.float32)
    cs2 = params["cg2"] * zk
    ct2 = -params["crm2"] * cs2 + params["cbe2"]
    cW2p = (cs1[:, None] * params["cW2"]).astype(np.float32)
    cb2p = (ct1 @ params["cW2"] + params["cb2"]).astype(np.float32)
    cW3p = (cs2[:, None] * params["cW3"]).astype(np.float32)
    cb3p = (ct2 @ params["cW3"] + params["cb3"]).astype(np.float32)

    vecs = np.zeros((P, 13), dtype=np.float32)
    vecs[:, 0], vecs[:, 1] = s1, t1
    vecs[:, 2], vecs[:, 3] = s2[:128], t2[:128]
    vecs[:, 4], vecs[:, 5] = s2[128:], t2[128:]
    vecs[:, 6], vecs[:, 7] = s3, t3
    vecs[:64, 8], vecs[:64, 9] = s4, t4
    vecs[:64, 10] = params["cb1"]
    vecs[:32, 11] = cb2p
    vecs[:2, 12] = cb3p

    return dict(
        N=N, SHARD=SHARD, TSHARD=TSHARD, NT=NT, TROWS=TROWS, CSH=CSH, RNGW=RNGW,
        TOT=TOT, uses=uses, n_uses=n_uses,
        blk_off=blk_off, blk_rows=blk_rows,
        idx_w=idx_w, dstl_w=dstl_w, dis_t=dis_t, xt=xt, vecs=vecs,
        W1=params["W1"].astype(np.float32), W2=params["W2"].astype(np.float32),
        W3=np.concatenate([params["W3"][:128], params["W3"][128:]], axis=1).astype(np.float32),
        W4=params["W4"].astype(np.float32),
        cW1=params["cW1"].astype(np.float32), cW2p=cW2p, cW3p=cW3p,
        d_in=x.shape[1],
    )


def _build(meta):
    """Build the Bass program (same for all cores)."""
    NT, TSHARD, TROWS = meta["NT"], meta["TSHARD"], meta["TROWS"]
    CSH, RNGW, TOT = meta["CSH"], meta["RNGW"], meta["TOT"]
    uses, n_uses = meta["uses"], meta["n_uses"]
    blk_off, blk_rows = meta["blk_off"], meta["blk_rows"]
    D_IN = meta["d_in"]
    CRNG = CSH * NDEV  # rows per chunk tensor (50176) = 2 ranges

    nc = bacc.Bacc(None, target_bir_lowering=False, num_swdge_queues=4)
    t_xt = [nc.dram_tensor(f"xt{c}", [CRNG, WTAB], bf16, kind="ExternalInput")
            for c in range(NCH)]
    t_idx = nc.dram_tensor("idx", [P, TOT // 16], i16, kind="ExternalInput")
    t_dstl = nc.dram_tensor("dstl", [P, n_uses], f32, kind="ExternalInput")
    t_xto = nc.dram_tensor("xt_own", [TSHARD, WTAB], bf16, kind="ExternalInput")
    t_dis = nc.dram_tensor("dis", [P, NT], f32, kind="ExternalInput")
    t_vecs = nc.dram_tensor("vecs", [P, 13], f32, kind="ExternalInput")
    t_W1 = nc.dram_tensor("W1", [D_IN, 128], f32, kind="ExternalInput")
    t_W2 = nc.dram_tensor("W2", [128, 256], f32, kind="ExternalInput")
    t_W3 = nc.dram_tensor("W3", [128, 256], f32, kind="ExternalInput")  # packed K-halves
    t_W4 = nc.dram_tensor("W4", [128, 64], f32, kind="ExternalInput")
    t_cW1 = nc.dram_tensor("cW1", [64, 64], f32, kind="ExternalInput")
    t_cW2 = nc.dram_tensor("cW2p", [64, 32], f32, kind="ExternalInput")
    t_cW3 = nc.dram_tensor("cW3p", [32, 2], f32, kind="ExternalInput")
    t_out = nc.dram_tensor("outT", [2, TSHARD], f32, kind="ExternalOutput")

    cc_in = [[nc.dram_tensor(f"cc_in{k}_{c}", [CSH, WTAB], bf16) for c in range(NCH)]
             for k in range(3)]
    tabs = [[nc.dram_tensor(f"tab{k}_{c}", [CRNG, WTAB], bf16, addr_space="Shared")
             for c in range(NCH)] for k in range(3)]

    qctr = [0]

    def qrr():
        qctr[0] = (qctr[0] + 1) % 4
        return qctr[0]

    with tile.TileContext(nc) as tc:
        with (
            tc.tile_pool(name="const", bufs=1) as cpool,
            tc.tile_pool(name="gp", bufs=15) as gpool,
            tc.tile_pool(name="sp", bufs=6) as spool,
            tc.tile_pool(name="yq", bufs=2) as ypool,
            tc.tile_pool(name="pagg", bufs=2, space="PSUM") as pagg,
            tc.tile_pool(name="paux", bufs=3, space="PSUM") as paux,
            tc.tile_pool(name="ep", bufs=3) as ep,
        ):
            # ---- constants
            idx_sb = cpool.tile([P, TOT // 16], i16)
            nc.sync.dma_start(out=idx_sb[:], in_=t_idx[:])
            dstlf_sb = cpool.tile([P, n_uses], f32)
            nc.sync.dma_start(out=dstlf_sb[:], in_=t_dstl[:])
            dstl_sb = cpool.tile([P, n_uses], bf16)
            nc.vector.tensor_copy(out=dstl_sb[:], in_=dstlf_sb[:])
            dstln_sb = cpool.tile([P, n_uses], f32)
            nc.vector.tensor_scalar_mul(dstln_sb[:], dstlf_sb[:], -1.0)
            dis_sb = cpool.tile([P, NT], f32)
            nc.sync.dma_start(out=dis_sb[:], in_=t_dis[:])
            vecs_sb = cpool.tile([P, 13], f32)
            nc.sync.dma_start(out=vecs_sb[:], in_=t_vecs[:])
            W1_sb = cpool.tile([D_IN, 128], f32)
            nc.sync.dma_start(out=W1_sb[:], in_=t_W1[:])
            W2_sb = cpool.tile([128, 256], f32)
            nc.sync.dma_start(out=W2_sb[:], in_=t_W2[:])
            W3_sb = cpool.tile([128, 256], f32)
            nc.sync.dma_start(out=W3_sb[:], in_=t_W3[:])
            W4_sb = cpool.tile([128, 64], f32)
            nc.sync.dma_start(out=W4_sb[:], in_=t_W4[:])
            cW1_sb = cpool.tile([64, 64], f32)
            nc.sync.dma_start(out=cW1_sb[:], in_=t_cW1[:])
            cW2_sb = cpool.tile([64, 32], f32)
            nc.sync.dma_start(out=cW2_sb[:], in_=t_cW2[:])
            cW3_sb = cpool.tile([32, 2], f32)
            nc.sync.dma_start(out=cW3_sb[:], in_=t_cW3[:])
            ident = cpool.tile([P, P], f32)
            make_identity(nc, ident[:])
            ident_bf = cpool.tile([P, P], bf16)
            nc.vector.tensor_copy(out=ident_bf[:], in_=ident[:])
            KMAX = max((len(uses[t][r]) for t in range(NT) for r in range(NR)),
                       default=1)
            iota_i = cpool.tile([P, KMAX, P], i32)
            nc.gpsimd.iota(iota_i[:], pattern=[[0, KMAX], [1, P]], base=0,
                           channel_multiplier=0)
            iota_bf = cpool.tile([P, KMAX, P], bf16)
            nc.vector.tensor_copy(out=iota_bf[:], in_=iota_i[:])

            AluEq = mybir.AluOpType.is_equal
            ACTF = mybir.ActivationFunctionType

            def transpose_f32(src_sb, pdim, fdim):
                """[pdim, fdim] f32 sbuf -> [fdim, pdim] f32 sbuf (PE transpose)."""
                tp = paux.tile([fdim, pdim], f32, tag="mm")
                nc.tensor.transpose(tp[:], src_sb[:], ident[:pdim, :pdim])
                out = ep.tile([fdim, pdim], f32, tag="tps")
                nc.vector.tensor_copy(out=out[:], in_=tp[:])
                return out

            def emit_ag(k, c):
                nc.gpsimd.collective_compute(
                    "AllGather", mybir.AluOpType.bypass,
                    replica_groups=[list(range(NDEV))],
                    ins=[cc_in[k][c][:]], outs=[tabs[k][c][:]],
                )

            def emit_gather(k, b, r):
                rows = int(blk_rows[b, r])
                if rows == 0:
                    return None
                g = gpool.tile([P, rows // P, WTAB], bf16, tag="g")
                off = int(blk_off[b, r])
                if k == 0:
                    table = t_xt[r // 2]
                else:
                    table = tabs[k - 1][r // 2]
                lo = (r % 2) * RNGW
                nc.gpsimd.dma_gather(
                    out_ap=g[:],
                    in_ap=table[lo:lo + RNGW, :],
                    idxs_ap=idx_sb[:, off // 16:(off + rows) // 16],
                    num_idxs=rows,
                    num_idxs_reg=rows,
                    elem_size=WTAB,
                    single_packet=False,
                    queue_num=qrr(),
                )
                return g

            def emit_tiles(k, b, gt, w, epilogue):
                """Aggregation matmuls + epilogue for block b (gt: r->gather)."""
                for t in range(b * BLKT, (b + 1) * BLKT):
                    own = ep.tile([P, WTAB], bf16, tag="own")
                    if k == 0:
                        src_ap = t_xto[t * P:(t + 1) * P, :]
                    else:
                        c, pt = t // CHT, t % CHT
                        src_ap = cc_in[k - 1][c][pt * P:(pt + 1) * P, :]
                    nc.sync.dma_start(out=own[:], in_=src_ap)
                    spt = {}
                    for r in range(NR):
                        ul = uses[t][r]
                        if not ul:
                            continue
                        du0 = ul[0][1]
                        sP = spool.tile([P, len(ul), P], bf16, tag="s")
                        if (t * NR + r) % 8 != 7:
                            nc.vector.tensor_tensor(
                                out=sP[:],
                                in0=dstl_sb[:, du0:du0 + len(ul)].to_broadcast([P, len(ul), P]),
                                in1=iota_bf[:, :len(ul), :],
                                op=AluEq,
                            )
                        else:
                            # ACT path: s = Relu(1 - (iota - dstl)^2)
                            yq = ypool.tile([P, len(ul), P], bf16, tag="yq")
                            for ui in range(len(ul)):
                                nc.scalar.activation(
                                    yq[:, ui, :], iota_bf[:, ui, :], ACTF.Square,
                                    bias=dstln_sb[:, du0 + ui:du0 + ui + 1])
                            nc.scalar.activation(sP[:], yq[:], ACTF.Relu,
                                                 bias=1.0, scale=-1.0)
                        spt[r] = sP
                    nmm = 1 + sum(len(uses[t][r]) for r in range(NR))
                    ps = pagg.tile([P, w], f32, tag="pagg")
                    nc.tensor.matmul(ps[:], lhsT=ident_bf[:], rhs=own[:, :w],
                                     start=True, stop=(nmm == 1))
                    kk = 1
                    for r in range(NR):
                        for ui, (lc, du, _ci) in enumerate(uses[t][r]):
                            nc.tensor.matmul(
                                ps[:], lhsT=spt[r][:, ui, :], rhs=gt[r][:, lc, :w],
                                start=False, stop=(kk == nmm - 1),
                            )
                            kk += 1
                    epilogue(t, ps)

            def phase(k, w, epilogue):
                """Emission schedule (see module docstring): chunk-1-range
                gathers of blocks 0..DEFER-1 are deferred to iterations
                DEFER..2*DEFER-1; AG[k-1][1] is emitted at iteration 1;
                AG[k][0] mid-phase once chunk-0 tiles are done."""
                gts = {}  # b -> {r: gather tile}
                for i in range(NBLK):
                    b = i
                    gts.setdefault(b, {})
                    if k > 0 and i == 1:
                        emit_ag(k - 1, 1)
                    # chunk-0 ranges of block b
                    for r in (0, 1):
                        gts[b][r] = emit_gather(k, b, r)
                    if b >= DEFER:
                        for r in (2, 3):
                            gts[b][r] = emit_gather(k, b, r)
                    # catch-up: deferred chunk-1 ranges of early blocks
                    if DEFER <= i < 2 * DEFER:
                        bd = i - DEFER
                        for r in (2, 3):
                            gts[bd][r] = emit_gather(k, bd, r)
                        emit_tiles(k, bd, gts.pop(bd), w, epilogue)
                    if b >= DEFER:
                        emit_tiles(k, b, gts.pop(b), w, epilogue)
                    if k < 3 and i == 9:
                        emit_ag(k, 0)  # chunk-0 tiles (b0..6) done by now

            def wr_cc(k, t, src):
                c, pt = t // CHT, t % CHT
                nc.sync.dma_start(out=cc_in[k][c][pt * P:(pt + 1) * P, :], in_=src)

            # ================= Phase 1: L1 =================
            def ep1(t, ps):
                a = ep.tile([P, D_IN], f32, tag="a1")
                nc.scalar.activation(a[:], ps[:], ACTF.Copy, scale=dis_sb[:, t:t + 1])
                aT = transpose_f32(a, P, D_IN)
                hps = paux.tile([128, P], f32, tag="mm")
                nc.tensor.matmul(hps[:], lhsT=W1_sb[:], rhs=aT[:], start=True, stop=True)
                hT = ep.tile([128, P], f32, tag="h1T")
                nc.scalar.activation(hT[:], hps[:], ACTF.Relu,
                                     bias=vecs_sb[:, 1:2], scale=vecs_sb[:, 0:1])
                hp = paux.tile([P, 128], f32, tag="mm")
                nc.tensor.transpose(hp[:], hT[:], ident[:])
                hb = ep.tile([P, WTAB], bf16, tag="h1b")
                nc.scalar.activation(hb[:], hp[:], ACTF.Copy, scale=dis_sb[:, t:t + 1])
                wr_cc(0, t, hb[:])

            phase(0, D_IN, ep1)

            # ================= Phase 2: L2 + dense L3 =================
            def ep2(t, ps):
                a = ep.tile([P, 128], f32, tag="a2")
                nc.scalar.activation(a[:], ps[:], ACTF.Copy, scale=dis_sb[:, t:t + 1])
                aT = transpose_f32(a, P, 128)
                y3ps = paux.tile([128, P], f32, tag="acc")
                for h in range(2):
                    hps = paux.tile([128, P], f32, tag="mm")
                    nc.tensor.matmul(hps[:], lhsT=W2_sb[:, h * 128:(h + 1) * 128],
                                     rhs=aT[:], start=True, stop=True)
                    hT = ep.tile([128, P], f32, tag="h2T")
                    nc.scalar.activation(hT[:], hps[:], ACTF.Relu,
                                         bias=vecs_sb[:, 3 + 2 * h:4 + 2 * h],
                                         scale=vecs_sb[:, 2 + 2 * h:3 + 2 * h])
                    nc.tensor.matmul(y3ps[:], lhsT=W3_sb[:, h * 128:(h + 1) * 128],
                                     rhs=hT[:], start=(h == 0), stop=(h == 1))
                y3T = ep.tile([128, P], f32, tag="y3T")
                nc.vector.tensor_copy(out=y3T[:], in_=y3ps[:])
                y3p = paux.tile([P, 128], f32, tag="mm")
                nc.tensor.transpose(y3p[:], y3T[:], ident[:])
                y3b = ep.tile([P, WTAB], bf16, tag="y3b")
                nc.scalar.activation(y3b[:], y3p[:], ACTF.Copy, scale=dis_sb[:, t:t + 1])
                wr_cc(1, t, y3b[:])

            phase(1, 128, ep2)

            # ================= Phase 3: L3 agg + dense L4 =================
            def ep3(t, ps):
                z = ep.tile([P, 128], f32, tag="z3")
                nc.scalar.activation(z[:], ps[:], ACTF.Copy, scale=dis_sb[:, t:t + 1])
                zT = transpose_f32(z, P, 128)
                h3T = ep.tile([128, P], f32, tag="h3T")
                nc.scalar.activation(h3T[:], zT[:], ACTF.Relu,
                                     bias=vecs_sb[:, 7:8], scale=vecs_sb[:, 6:7])
                y4ps = paux.tile([64, P], f32, tag="mm")
                nc.tensor.matmul(y4ps[:], lhsT=W4_sb[:], rhs=h3T[:], start=True, stop=True)
                y4T = ep.tile([64, P], f32, tag="y4T")
                nc.vector.tensor_copy(out=y4T[:], in_=y4ps[:])
                y4p = paux.tile([P, 64], f32, tag="mm")
                nc.tensor.transpose(y4p[:], y4T[:], ident[:64, :64])
                y4b = ep.tile([P, WTAB], bf16, tag="y4b")
                nc.vector.memset(y4b[:, 64:], 0)
                nc.scalar.activation(y4b[:, :64], y4p[:], ACTF.Copy,
                                     scale=dis_sb[:, t:t + 1])
                wr_cc(2, t, y4b[:])

            phase(2, 128, ep3)

            # ================= Phase 4: L4 agg + classifier =================
            def ep4(t, ps):
                z = ep.tile([P, 64], f32, tag="z4")
                nc.scalar.activation(z[:], ps[:], ACTF.Copy, scale=dis_sb[:, t:t + 1])
                zT = transpose_f32(z, P, 64)
                h4T = ep.tile([64, P], f32, tag="h4T")
                nc.scalar.activation(h4T[:], zT[:], ACTF.Relu,
                                     bias=vecs_sb[:64, 9:10], scale=vecs_sb[:64, 8:9])
                u1ps = paux.tile([64, P], f32, tag="mm")
                nc.tensor.matmul(u1ps[:], lhsT=cW1_sb[:], rhs=h4T[:], start=True, stop=True)
                u1T = ep.tile([64, P], f32, tag="u1T")
                nc.scalar.activation(u1T[:], u1ps[:], ACTF.Relu, bias=vecs_sb[:64, 10:11])
                u2ps = paux.tile([32, P], f32, tag="mm")
                nc.tensor.matmul(u2ps[:], lhsT=cW2_sb[:], rhs=u1T[:], start=True, stop=True)
                u2T = ep.tile([32, P], f32, tag="u2T")
                nc.scalar.activation(u2T[:], u2ps[:], ACTF.Relu, bias=vecs_sb[:32, 11:12])
                ops_ = paux.tile([2, P], f32, tag="mm")
                nc.tensor.matmul(ops_[:], lhsT=cW3_sb[:], rhs=u2T[:], start=True, stop=True)
                oT = ep.tile([2, P], f32, tag="oT")
                nc.scalar.activation(oT[:], ops_[:], ACTF.Identity, bias=vecs_sb[:2, 12:13])
                nc.sync.dma_start(out=t_out[:, t * P:(t + 1) * P], in_=oT[:])

            phase(3, 64, ep4)

    nc.finalize()
    return nc


_CACHE = {}


def kernel(**inputs):
    x = np.asarray(inputs["x"], dtype=np.float32)
    edge_index = np.asarray(inputs["edge_index"])
    N = x.shape[0]
    key = hashlib.sha256(edge_index.tobytes()).hexdigest()[:16] + f"_{N}_{x.shape[1]}"
    if key not in _CACHE:
        meta = _prep(x, edge_index, inputs, N)
        nc = _build(meta)
        _CACHE[key] = (meta, nc)
    else:
        meta, nc = _CACHE[key]
        # x may differ between calls with same graph: recompute xt
        meta = dict(meta)
        m2 = _prep(x, edge_index, inputs, N)
        meta["xt"] = m2["xt"]
        meta.update({k: m2[k] for k in ("vecs", "W1", "W2", "W3", "W4", "cW1", "cW2p", "cW3p", "dis_t")})

    CSH = meta["CSH"]
    CRNG = CSH * NDEV
    in_maps = []
    for d in range(NDEV):
        im = {
            "idx": meta["idx_w"][d],
            "dstl": meta["dstl_w"][d],
            "dis": meta["dis_t"][d],
            "vecs": meta["vecs"],
            "W1": meta["W1"], "W2": meta["W2"], "W3": meta["W3"], "W4": meta["W4"],
            "cW1": meta["cW1"], "cW2p": meta["cW2p"], "cW3p": meta["cW3p"],
        }
        for c in range(NCH):
            im[f"xt{c}"] = meta["xt"][c * CRNG:(c + 1) * CRNG]
        im["xt_own"] = np.concatenate([
            meta["xt"][c * CRNG + d * CSH:c * CRNG + (d + 1) * CSH]
            for c in range(NCH)
        ])
        in_maps.append(im)
    res = None
    for _attempt in range(4):
        try:
            res = run_bass_kernel_spmd(nc, in_maps, core_ids=list(range(NDEV)), trace=False)
            break
        except Exception:
            if _attempt == 3:
                raise

    SHARD = meta["SHARD"]
    out = np.empty((N, 2), dtype=np.float32)
    for d in range(NDEV):
        out[d * SHARD:(d + 1) * SHARD] = res.results[d]["outT"][:, :SHARD].T
    return out


# revision 10
# speedup vs baseline: 1.8937x; 1.0198x over previous
"""Trainium2 Bass kernel for nn_EnhancedGCN42 (4-layer GCN + MLP classifier).

Strategy (8 NeuronCores, SPMD single NEFF):
  - Nodes dst-sharded: device d owns dst nodes [d*12500, (d+1)*12500).
  - A-hat = D^-1/2 (A+I) D^-1/2 factorized: tables store dis*h rows (bf16,
    256B rows); aggregation output scaled by dis_dst.
  - Row space is chunk-major with 2 chunks of 49 tiles per device. Each
    phase's table lives in 2 DRAM tensors (one per chunk) so collective ->
    gather deps are exact per chunk. AllGathers have a large (~100us)
    fixed cost, so only 2 per phase; their latency is hidden by emission
    scheduling: chunk-0's AG fires mid-phase (once its 49 tiles are done),
    chunk-1's AG fires at the start of the next phase, and the next
    phase's gathers are ordered so chunk-1-range gathers of the first 4
    blocks are deferred (catch-up at iterations 4..7) until that AG has
    landed. The gpsimd stream (gather descriptor generation, the
    critical resource) then never head-of-line blocks on collectives.
  - Per layer: per-edge rows gathered via dma_gather (4 SWDGE queues),
    aggregated per 128-dst tile by matmul with an on-chip-built one-hot
    selection matrix (is_equal against iota on vector, 1/8 offloaded to
    scalar via a Relu(1-(iota-dstl)^2) trick).
  - Dense W / BN / ReLU fused per dst-tile in transposed layout; BN and
    classifier BN folded on host into per-feature scale/bias.

kernel(**inputs) -> [100000, 2] float32.
"""
import hashlib
import numpy as np
import ml_dtypes

import concourse.bacc as bacc
import concourse.bass as bass
import concourse.mybir as mybir
import concourse.tile as tile
from concourse.masks import make_identity
from concourse.bass_utils import run_bass_kernel_spmd

f32 = mybir.dt.float32
bf16 = mybir.dt.bfloat16
i16 = mybir.dt.int16
i32 = mybir.dt.int32
nbf16 = ml_dtypes.bfloat16

P = 128
NDEV = 8
EPS = 1e-5
WTAB = 128       # table row = 128 cols bf16 = 256B
NCH = 2          # allgather chunks per phase
CHT = 49         # tiles per chunk
NR = 4           # gather src ranges (2 per chunk; int16 limit 25088<=32768)
BLKT = 7         # tiles per gather block
NBLK = 14        # blocks per phase
DEFER = 4        # blocks whose chunk-1-range gathers are deferred


def _prep(x, edge_index, params, N):
    """Host preprocessing: graph partition + folded constants. Returns meta dict."""
    SHARD = N // NDEV                       # 12500
    TSHARD = ((SHARD + P - 1) // P) * P     # 12544
    NT = TSHARD // P                        # 98
    TROWS = TSHARD * NDEV                   # 100352
    CSH = CHT * P                           # 6272 rows per device per chunk
    RNGW = TROWS // NR                      # 25088 rows per range
    assert NT == NCH * CHT == NBLK * BLKT and RNGW <= 32768

    ei = edge_index.astype(np.int64)
    loop = np.arange(N, dtype=np.int64)
    dst_all = np.concatenate([ei[1], loop])
    deg = np.bincount(dst_all, minlength=N).astype(np.float32)
    dis = (1.0 / np.sqrt(deg)).astype(np.float32)

    def padrow(n):
        # node -> chunk-major padded row: chunk c spans all devices' c-th
        # 49-tile sub-shard. row = c*CSH*NDEV + d*CSH + (local - c*CSH)
        d = n // SHARD
        local = n - d * SHARD
        c = local // CSH
        return c * CSH * NDEV + d * CSH + (local - c * CSH)

    src_e = ei[0]
    dst_e = ei[1]
    psrc_e = padrow(src_e)

    counts = np.zeros((NDEV, NT, NR), dtype=np.int64)
    dev_edges = []
    for d in range(NDEV):
        m = (dst_e >= d * SHARD) & (dst_e < (d + 1) * SHARD)
        es = psrc_e[m]
        el = dst_e[m] - d * SHARD
        t_id = el >> 7
        r_id = es // RNGW
        order = np.lexsort((es, r_id, t_id))  # (tile, range, src-ascending)
        es, el, t_id, r_id = es[order], el[order], t_id[order], r_id[order]
        np.add.at(counts[d], (t_id, r_id), 1)
        dev_edges.append((es, el))

    grp_rows = counts.max(axis=0).astype(np.int64)  # [NT, NR] exact max

    grp_off = np.zeros((NT, NR), dtype=np.int64)
    blk_off = np.zeros((NBLK, NR), dtype=np.int64)
    blk_rows = np.zeros((NBLK, NR), dtype=np.int64)
    acc = 0
    for b in range(NBLK):
        for r in range(NR):
            blk_off[b, r] = acc
            for t in range(b * BLKT, (b + 1) * BLKT):
                grp_off[t, r] = acc
                acc += grp_rows[t, r]
            acc = (acc + P - 1) // P * P  # pad gather to whole chunks
            blk_rows[b, r] = acc - blk_off[b, r]
    TOT = acc
    assert TOT % 16 == 0

    # chunk-use enumeration: per (t, r) the 128-row chunks its group overlaps.
    uses = [[[] for _ in range(NR)] for _ in range(NT)]  # (local_col, dstl_col)
    n_uses = 0
    for b in range(NBLK):
        for r in range(NR):
            for t in range(b * BLKT, (b + 1) * BLKT):
                g0, g1 = grp_off[t, r], grp_off[t, r] + grp_rows[t, r]
                if g1 == g0:
                    continue
                c0, c1 = int(g0 // P), int((g1 + P - 1) // P)
                for ci in range(c0, c1):
                    uses[t][r].append((ci - int(blk_off[b, r]) // P, n_uses, ci))
                    n_uses += 1

    idx_w = np.zeros((NDEV, P, TOT // 16), dtype=np.int16)
    dstl_w = np.full((NDEV, P, n_uses), 255.0, dtype=np.float32)
    for d in range(NDEV):
        es, el = dev_edges[d]
        IDX = np.zeros(TOT, dtype=np.int16)
        DLOC = np.full(TOT, 255.0, dtype=np.float32)
        OWNER = np.full(TOT, -1, dtype=np.int64)
        pos = 0
        for t in range(NT):
            for r in range(NR):
                c = int(counts[d, t, r])
                o = int(grp_off[t, r])
                IDX[o:o + c] = (es[pos:pos + c] - r * RNGW).astype(np.int16)
                DLOC[o:o + c] = (el[pos:pos + c] - t * P).astype(np.float32)
                OWNER[o:o + int(grp_rows[t, r])] = t
                pos += c
        idx_w[d] = np.tile(IDX.reshape(-1, 16).T, (8, 1))
        D = dstl_w[d]
        for t in range(NT):
            for r in range(NR):
                for (_lc, du, ci) in uses[t][r]:
                    rows = np.arange(ci * P, (ci + 1) * P)
                    v = np.where(OWNER[rows] == t, DLOC[rows], 255.0)
                    D[:, du] = v

    dis_pad = np.zeros(TROWS, dtype=np.float32)
    dis_pad[padrow(np.arange(N))] = dis
    dis_t = np.zeros((NDEV, P, NT), dtype=np.float32)
    for d in range(NDEV):
        for t in range(NT):
            c = t // CHT
            g0 = c * CSH * NDEV + d * CSH + (t - c * CHT) * P
            dis_t[d, :, t] = dis_pad[g0:g0 + P]

    # x-tilde table (bf16, padded cols), chunk-major row space
    xt = np.zeros((TROWS, WTAB), dtype=nbf16)
    v = (dis[:, None] * x).astype(nbf16)
    xt[padrow(np.arange(N)), :x.shape[1]] = v

    def fold(g, be, rm, rv, b):
        k = (1.0 / np.sqrt(rv + EPS)).astype(np.float32)
        s = g * k
        t = (b - rm) * s + be
        return s.astype(np.float32), t.astype(np.float32)

    s1, t1 = fold(params["g1"], params["be1"], params["rm1"], params["rv1"], params["b1"])
    s2, t2 = fold(params["g2"], params["be2"], params["rm2"], params["rv2"], params["b2"])
    s3, t3 = fold(params["g3"], params["be3"], params["rm3"], params["rv3"], params["b3"])
    s4, t4 = fold(params["g4"], params["be4"], params["rm4"], params["rv4"], params["b4"])
    zk = (1.0 / np.sqrt(params["crv1"] + EPS)).astype(np.float32)
    cs1 = params["cg1"] * zk
    ct1 = -params["crm1"] * cs1 + params["cbe1"]
    zk = (1.0 / np.sqrt(params["crv2"] + EPS)).astype(np.float32)
    cs2 = params["cg2"] * zk
    ct2 = -params["crm2"] * cs2 + params["cbe2"]
    cW2p = (cs1[:, None] * params["cW2"]).astype(np.float32)
    cb2p = (ct1 @ params["cW2"] + params["cb2"]).astype(np.float32)
    cW3p = (cs2[:, None] * params["cW3"]).astype(np.float32)
    cb3p = (ct2 @ params["cW3"] + params["cb3"]).astype(np.float32)

    vecs = np.zeros((P, 13), dtype=np.float32)
    vecs[:, 0], vecs[:, 1] = s1, t1
    vecs[:, 2], vecs[:, 3] = s2[:128], t2[:128]
    vecs[:, 4], vecs[:, 5] = s2[128:], t2[128:]
    vecs[:, 6], vecs[:, 7] = s3, t3
    vecs[:64, 8], vecs[:64, 9] = s4, t4
    vecs[:64, 10] = params["cb1"]
    vecs[:32, 11] = cb2p
    vecs[:2, 12] = cb3p

    return dict(
        N=N, SHARD=SHARD, TSHARD=TSHARD, NT=NT, TROWS=TROWS, CSH=CSH, RNGW=RNGW,
        TOT=TOT, uses=uses, n_uses=n_uses,
        blk_off=blk_off, blk_rows=blk_rows,
        idx_w=idx_w, dstl_w=dstl_w, dis_t=dis_t, xt=xt, vecs=vecs,
        W1=params["W1"].astype(np.float32), W2=params["W2"].astype(np.float32),
        W3=np.concatenate([params["W3"][:128], params["W3"][128:]], axis=1).astype(np.float32),
        W4=params["W4"].astype(np.float32),
        cW1=params["cW1"].astype(np.float32), cW2p=cW2p, cW3p=cW3p,
        d_in=x.shape[1],
    )


def _build(meta):
    """Build the Bass program (same for all cores)."""
    NT, TSHARD, TROWS = meta["NT"], meta["TSHARD"], meta["TROWS"]
    CSH, RNGW, TOT = meta["CSH"], meta["RNGW"], meta["TOT"]
    uses, n_uses = meta["uses"], meta["n_uses"]
    blk_off, blk_rows = meta["blk_off"], meta["blk_rows"]
    D_IN = meta["d_in"]
    CRNG = CSH * NDEV  # rows per chunk tensor (50176) = 2 ranges

    nc = bacc.Bacc(None, target_bir_lowering=False, num_swdge_queues=4)
    t_xt = [nc.dram_tensor(f"xt{c}", [CRNG, WTAB], bf16, kind="ExternalInput")
            for c in range(NCH)]
    t_idx = nc.dram_tensor("idx", [P, TOT // 16], i16, kind="ExternalInput")
    t_dstl = nc.dram_tensor("dstl", [P, n_uses], f32, kind="ExternalInput")
    t_xto = nc.dram_tensor("xt_own", [TSHARD, WTAB], bf16, kind="ExternalInput")
    t_dis = nc.dram_tensor("dis", [P, NT], f32, kind="ExternalInput")
    t_vecs = nc.dram_tensor("vecs", [P, 13], f32, kind="ExternalInput")
    t_W1 = nc.dram_tensor("W1", [D_IN, 128], f32, kind="ExternalInput")
    t_W2 = nc.dram_tensor("W2", [128, 256], f32, kind="ExternalInput")
    t_W3 = nc.dram_tensor("W3", [128, 256], f32, kind="ExternalInput")  # packed K-halves
    t_W4 = nc.dram_tensor("W4", [128, 64], f32, kind="ExternalInput")
    t_cW1 = nc.dram_tensor("cW1", [64, 64], f32, kind="ExternalInput")
    t_cW2 = nc.dram_tensor("cW2p", [64, 32], f32, kind="ExternalInput")
    t_cW3 = nc.dram_tensor("cW3p", [32, 2], f32, kind="ExternalInput")
    t_out = nc.dram_tensor("outT", [2, TSHARD], f32, kind="ExternalOutput")

    cc_in = [[nc.dram_tensor(f"cc_in{k}_{c}", [CSH, WTAB], bf16) for c in range(NCH)]
             for k in range(3)]
    tabs = [[nc.dram_tensor(f"tab{k}_{c}", [CRNG, WTAB], bf16, addr_space="Shared")
             for c in range(NCH)] for k in range(3)]

    qctr = [0]

    def qrr():
        qctr[0] = (qctr[0] + 1) % 4
        return qctr[0]

    with tile.TileContext(nc) as tc:
        with (
            tc.tile_pool(name="const", bufs=1) as cpool,
            tc.tile_pool(name="gp", bufs=15) as gpool,
            tc.tile_pool(name="sp", bufs=6) as spool,
            tc.tile_pool(name="yq", bufs=2) as ypool,
            tc.tile_pool(name="pagg", bufs=2, space="PSUM") as pagg,
            tc.tile_pool(name="paux", bufs=3, space="PSUM") as paux,
            tc.tile_pool(name="ep", bufs=3) as ep,
        ):
            # ---- constants
            idx_sb = cpool.tile([P, TOT // 16], i16)
            nc.sync.dma_start(out=idx_sb[:], in_=t_idx[:])
            dstlf_sb = cpool.tile([P, n_uses], f32)
            nc.sync.dma_start(out=dstlf_sb[:], in_=t_dstl[:])
            dstl_sb = cpool.tile([P, n_uses], bf16)
            nc.vector.tensor_copy(out=dstl_sb[:], in_=dstlf_sb[:])
            dstln_sb = cpool.tile([P, n_uses], f32)
            nc.vector.tensor_scalar_mul(dstln_sb[:], dstlf_sb[:], -1.0)
            dis_sb = cpool.tile([P, NT], f32)
            nc.sync.dma_start(out=dis_sb[:], in_=t_dis[:])
            vecs_sb = cpool.tile([P, 13], f32)
            nc.sync.dma_start(out=vecs_sb[:], in_=t_vecs[:])
            W1_sb = cpool.tile([D_IN, 128], f32)
            nc.sync.dma_start(out=W1_sb[:], in_=t_W1[:])
            W2_sb = cpool.tile([128, 256], f32)
            nc.sync.dma_start(out=W2_sb[:], in_=t_W2[:])
            W3_sb = cpool.tile([128, 256], f32)
            nc.sync.dma_start(out=W3_sb[:], in_=t_W3[:])
            W4_sb = cpool.tile([128, 64], f32)
            nc.sync.dma_start(out=W4_sb[:], in_=t_W4[:])
            cW1_sb = cpool.tile([64, 64], f32)
            nc.sync.dma_start(out=cW1_sb[:], in_=t_cW1[:])
            cW2_sb = cpool.tile([64, 32], f32)
            nc.sync.dma_start(out=cW2_sb[:], in_=t_cW2[:])
            cW3_sb = cpool.tile([32, 2], f32)
            nc.sync.dma_start(out=cW3_sb[:], in_=t_cW3[:])
            ident = cpool.tile([P, P], f32)
            make_identity(nc, ident[:])
            ident_bf = cpool.tile([P, P], bf16)
            nc.vector.tensor_copy(out=ident_bf[:], in_=ident[:])
            KMAX = max((len(uses[t][r]) for t in range(NT) for r in range(NR)),
                       default=1)
            iota_i = cpool.tile([P, KMAX, P], i32)
            nc.gpsimd.iota(iota_i[:], pattern=[[0, KMAX], [1, P]], base=0,
                           channel_multiplier=0)
            iota_bf = cpool.tile([P, KMAX, P], bf16)
            nc.vector.tensor_copy(out=iota_bf[:], in_=iota_i[:])

            AluEq = mybir.AluOpType.is_equal
            ACTF = mybir.ActivationFunctionType

            def transpose_f32(src_sb, pdim, fdim):
                """[pdim, fdim] f32 sbuf -> [fdim, pdim] f32 sbuf (PE transpose)."""
                tp = paux.tile([fdim, pdim], f32, tag="mm")
                nc.tensor.transpose(tp[:], src_sb[:], ident[:pdim, :pdim])
                out = ep.tile([fdim, pdim], f32, tag="tps")
                nc.vector.tensor_copy(out=out[:], in_=tp[:])
                return out

            def emit_ag(k, c):
                nc.gpsimd.collective_compute(
                    "AllGather", mybir.AluOpType.bypass,
                    replica_groups=[list(range(NDEV))],
                    ins=[cc_in[k][c][:]], outs=[tabs[k][c][:]],
                )

            def emit_gather(k, b, r):
                rows = int(blk_rows[b, r])
                if rows == 0:
                    return None
                g = gpool.tile([P, rows // P, WTAB], bf16, tag="g")
                off = int(blk_off[b, r])
                if k == 0:
                    table = t_xt[r // 2]
                else:
                    table = tabs[k - 1][r // 2]
                lo = (r % 2) * RNGW
                nc.gpsimd.dma_gather(
                    out_ap=g[:],
                    in_ap=table[lo:lo + RNGW, :],
                    idxs_ap=idx_sb[:, off // 16:(off + rows) // 16],
                    num_idxs=rows,
                    num_idxs_reg=rows,
                    elem_size=WTAB,
                    single_packet=False,
                    queue_num=qrr(),
                )
                return g

            def emit_tiles(k, b, gt, w, epilogue):
                """Aggregation matmuls + epilogue for block b (gt: r->gather)."""
                for t in range(b * BLKT, (b + 1) * BLKT):
                    own = ep.tile([P, WTAB], bf16, tag="own")
                    if k == 0:
                        src_ap = t_xto[t * P:(t + 1) * P, :]
                    else:
                        c, pt = t // CHT, t % CHT
                        src_ap = cc_in[k - 1][c][pt * P:(pt + 1) * P, :]
                    nc.sync.dma_start(out=own[:], in_=src_ap)
                    spt = {}
                    for r in range(NR):
                        ul = uses[t][r]
                        if not ul:
                            continue
                        du0 = ul[0][1]
                        sP = spool.tile([P, len(ul), P], bf16, tag="s")
                        if (t * NR + r) % 8 != 7:
                            nc.vector.tensor_tensor(
                                out=sP[:],
                                in0=dstl_sb[:, du0:du0 + len(ul)].to_broadcast([P, len(ul), P]),
                                in1=iota_bf[:, :len(ul), :],
                                op=AluEq,
                            )
                        else:
                            # ACT path: s = Relu(1 - (iota - dstl)^2)
                            yq = ypool.tile([P, len(ul), P], bf16, tag="yq")
                            for ui in range(len(ul)):
                                nc.scalar.activation(
                                    yq[:, ui, :], iota_bf[:, ui, :], ACTF.Square,
                                    bias=dstln_sb[:, du0 + ui:du0 + ui + 1])
                            nc.scalar.activation(sP[:], yq[:], ACTF.Relu,
                                                 bias=1.0, scale=-1.0)
                        spt[r] = sP
                    nmm = 1 + sum(len(uses[t][r]) for r in range(NR))
                    ps = pagg.tile([P, w], f32, tag="pagg")
                    nc.tensor.matmul(ps[:], lhsT=ident_bf[:], rhs=own[:, :w],
                                     start=True, stop=(nmm == 1))
                    kk = 1
                    for r in range(NR):
                        for ui, (lc, du, _ci) in enumerate(uses[t][r]):
                            nc.tensor.matmul(
                                ps[:], lhsT=spt[r][:, ui, :], rhs=gt[r][:, lc, :w],
                                start=False, stop=(kk == nmm - 1),
                            )
                            kk += 1
                    epilogue(t, ps)

            def phase(k, w, epilogue):
                """Emission schedule (see module docstring): chunk-1-range
                gathers of blocks 0..DEFER-1 are deferred to iterations
                DEFER..2*DEFER-1; AG[k-1][1] is emitted at iteration 1;
                AG[k][0] mid-phase once chunk-0 tiles are done."""
                gts = {}  # b -> {r: gather tile}
                for i in range(NBLK):
                    b = i
                    gts.setdefault(b, {})
                    if k > 0 and i == 3:
                        emit_ag(k - 1, 1)
                    # chunk-0 ranges of block b
                    for r in (0, 1):
                        gts[b][r] = emit_gather(k, b, r)
                    if b >= DEFER:
                        for r in (2, 3):
                            gts[b][r] = emit_gather(k, b, r)
                    # catch-up: deferred chunk-1 ranges of early blocks
                    if DEFER <= i < 2 * DEFER:
                        bd = i - DEFER
                        for r in (2, 3):
                            gts[bd][r] = emit_gather(k, bd, r)
                        emit_tiles(k, bd, gts.pop(bd), w, epilogue)
                    if b >= DEFER:
                        emit_tiles(k, b, gts.pop(b), w, epilogue)
                    if k < 3 and i == 9:
                        emit_ag(k, 0)  # chunk-0 tiles (b0..6) done by now

            def wr_cc(k, t, src):
                c, pt = t // CHT, t % CHT
                nc.sync.dma_start(out=cc_in[k][c][pt * P:(pt + 1) * P, :], in_=src)

            # ================= Phase 1: L1 =================
            def ep1(t, ps):
                a = ep.tile([P, D_IN], f32, tag="a1")
                nc.scalar.activation(a[:], ps[:], ACTF.Copy, scale=dis_sb[:, t:t + 1])
                aT = transpose_f32(a, P, D_IN)
                hps = paux.tile([128, P], f32, tag="mm")
                nc.tensor.matmul(hps[:], lhsT=W1_sb[:], rhs=aT[:], start=True, stop=True)
                hT = ep.tile([128, P], f32, tag="h1T")
                nc.scalar.activation(hT[:], hps[:], ACTF.Relu,
                                     bias=vecs_sb[:, 1:2], scale=vecs_sb[:, 0:1])
                hp = paux.tile([P, 128], f32, tag="mm")
                nc.tensor.transpose(hp[:], hT[:], ident[:])
                hb = ep.tile([P, WTAB], bf16, tag="h1b")
                nc.scalar.activation(hb[:], hp[:], ACTF.Copy, scale=dis_sb[:, t:t + 1])
                wr_cc(0, t, hb[:])

            phase(0, D_IN, ep1)

            # ================= Phase 2: L2 + dense L3 =================
            def ep2(t, ps):
                a = ep.tile([P, 128], f32, tag="a2")
                nc.scalar.activation(a[:], ps[:], ACTF.Copy, scale=dis_sb[:, t:t + 1])
                aT = transpose_f32(a, P, 128)
                y3ps = paux.tile([128, P], f32, tag="acc")
                for h in range(2):
                    hps = paux.tile([128, P], f32, tag="mm")
                    nc.tensor.matmul(hps[:], lhsT=W2_sb[:, h * 128:(h + 1) * 128],
                                     rhs=aT[:], start=True, stop=True)
                    hT = ep.tile([128, P], f32, tag="h2T")
                    nc.scalar.activation(hT[:], hps[:], ACTF.Relu,
                                         bias=vecs_sb[:, 3 + 2 * h:4 + 2 * h],
                                         scale=vecs_sb[:, 2 + 2 * h:3 + 2 * h])
                    nc.tensor.matmul(y3ps[:], lhsT=W3_sb[:, h * 128:(h + 1) * 128],
                                     rhs=hT[:], start=(h == 0), stop=(h == 1))
                y3T = ep.tile([128, P], f32, tag="y3T")
                nc.vector.tensor_copy(out=y3T[:], in_=y3ps[:])
                y3p = paux.tile([P, 128], f32, tag="mm")
                nc.tensor.transpose(y3p[:], y3T[:], ident[:])
                y3b = ep.tile([P, WTAB], bf16, tag="y3b")
                nc.scalar.activation(y3b[:], y3p[:], ACTF.Copy, scale=dis_sb[:, t:t + 1])
                wr_cc(1, t, y3b[:])

            phase(1, 128, ep2)

            # ================= Phase 3: L3 agg + dense L4 =================
            def ep3(t, ps):
                z = ep.tile([P, 128], f32, tag="z3")
                nc.scalar.activation(z[:], ps[:], ACTF.Copy, scale=dis_sb[:, t:t + 1])
                zT = transpose_f32(z, P, 128)
                h3T = ep.tile([128, P], f32, tag="h3T")
                nc.scalar.activation(h3T[:], zT[:], ACTF.Relu,
                                     bias=vecs_sb[:, 7:8], scale=vecs_sb[:, 6:7])
                y4ps = paux.tile([64, P], f32, tag="mm")
                nc.tensor.matmul(y4ps[:], lhsT=W4_sb[:], rhs=h3T[:], start=True, stop=True)
                y4T = ep.tile([64, P], f32, tag="y4T")
                nc.vector.tensor_copy(out=y4T[:], in_=y4ps[:])
                y4p = paux.tile([P, 64], f32, tag="mm")
                nc.tensor.transpose(y4p[:], y4T[:], ident[:64, :64])
                y4b = ep.tile([P, WTAB], bf16, tag="y4b")
                nc.vector.memset(y4b[:, 64:], 0)
                nc.scalar.activation(y4b[:, :64], y4p[:], ACTF.Copy,
                                     scale=dis_sb[:, t:t + 1])
                wr_cc(2, t, y4b[:])

            phase(2, 128, ep3)

            # ================= Phase 4: L4 agg + classifier =================
            def ep4(t, ps):
                z = ep.tile([P, 64], f32, tag="z4")
                nc.scalar.activation(z[:], ps[:], ACTF.Copy, scale=dis_sb[:, t:t + 1])
                zT = transpose_f32(z, P, 64)
                h4T = ep.tile([64, P], f32, tag="h4T")
                nc.scalar.activation(h4T[:], zT[:], ACTF.Relu,
                                     bias=vecs_sb[:64, 9:10], scale=vecs_sb[:64, 8:9])
                u1ps = paux.tile([64, P], f32, tag="mm")
                nc.tensor.matmul(u1ps[:], lhsT=cW1_sb[:], rhs=h4T[:], start=True, stop=True)
                u1T = ep.tile([64, P], f32, tag="u1T")
                nc.scalar.activation(u1T[:], u1ps[:], ACTF.Relu, bias=vecs_sb[:64, 10:11])
                u2ps = paux.tile([32, P], f32, tag="mm")
                nc.tensor.matmul(u2ps[:], lhsT=cW2_sb[:], rhs=u1T[:], start=True, stop=True)
                u2T = ep.tile([32, P], f32, tag="u2T")
                nc.scalar.activation(u2T[:], u2ps[:], ACTF.Relu, bias=vecs_sb[:32, 11:12])
                ops_ = paux.tile([2, P], f32, tag="mm")
                nc.tensor.matmul(ops_[:], lhsT=cW3_sb[:], rhs=u2T[:], start=True, stop=True)
                oT = ep.tile([2, P], f32, tag="oT")
                nc.scalar.activation(oT[:], ops_[:], ACTF.Identity, bias=vecs_sb[:2, 12:13])
                nc.sync.dma_start(out=t_out[:, t * P:(t + 1) * P], in_=oT[:])

            phase(3, 64, ep4)

    nc.finalize()
    return nc


_CACHE = {}


def kernel(**inputs):
    x = np.asarray(inputs["x"], dtype=np.float32)
    edge_index = np.asarray(inputs["edge_index"])
    N = x.shape[0]
    key = hashlib.sha256(edge_index.tobytes()).hexdigest()[:16] + f"_{N}_{x.shape[1]}"
    if key not in _CACHE:
        meta = _prep(x, edge_index, inputs, N)
        nc = _build(meta)
        _CACHE[key] = (meta, nc)
    else:
        meta, nc = _CACHE[key]
        # x may differ between calls with same graph: recompute xt
        meta = dict(meta)
        m2 = _prep(x, edge_index, inputs, N)
        meta["xt"] = m2["xt"]
        meta.update({k: m2[k] for k in ("vecs", "W1", "W2", "W3", "W4", "cW1", "cW2p", "cW3p", "dis_t")})

    CSH = meta["CSH"]
    CRNG = CSH * NDEV
    in_maps = []
    for d in range(NDEV):
        im = {
            "idx": meta["idx_w"][d],
            "dstl": meta["dstl_w"][d],
            "dis": meta["dis_t"][d],
            "vecs": meta["vecs"],
            "W1": meta["W1"], "W2": meta["W2"], "W3": meta["W3"], "W4": meta["W4"],
            "cW1": meta["cW1"], "cW2p": meta["cW2p"], "cW3p": meta["cW3p"],
        }
        for c in range(NCH):
            im[f"xt{c}"] = meta["xt"][c * CRNG:(c + 1) * CRNG]
        im["xt_own"] = np.concatenate([
            meta["xt"][c * CRNG + d * CSH:c * CRNG + (d + 1) * CSH]
            for c in range(NCH)
        ])
        in_maps.append(im)
    res = None
    for _attempt in range(4):
        try:
            res = run_bass_kernel_spmd(nc, in_maps, core_ids=list(range(NDEV)), trace=False)
            break
        except Exception:
            if _attempt == 3:
                raise

    SHARD = meta["SHARD"]
    out = np.empty((N, 2), dtype=np.float32)
    for d in range(NDEV):
        out[d * SHARD:(d + 1) * SHARD] = res.results[d]["outT"][:, :SHARD].T
    return out


# revision 17
# speedup vs baseline: 1.9254x; 1.0168x over previous
"""Trainium2 Bass kernel for nn_EnhancedGCN42 (4-layer GCN + MLP classifier).

Strategy (8 NeuronCores, SPMD single NEFF):
  - Nodes dst-sharded: device d owns dst nodes [d*12500, (d+1)*12500).
  - A-hat = D^-1/2 (A+I) D^-1/2 factorized: tables store dis*h rows (bf16,
    256B rows); aggregation output scaled by dis_dst.
  - Row space is chunk-major with 2 chunks of 49 tiles per device. Each
    phase's table lives in 2 DRAM tensors (one per chunk) so collective ->
    gather deps are exact per chunk. AllGathers have a large (~100us)
    fixed cost, so only 2 per phase; their latency is hidden by emission
    scheduling: chunk-0's AG fires mid-phase (once its 49 tiles are done),
    chunk-1's AG fires at the start of the next phase, and the next
    phase's gathers are ordered so chunk-1-range gathers of the first 4
    blocks are deferred (catch-up at iterations 4..7) until that AG has
    landed. The gpsimd stream (gather descriptor generation, the
    critical resource) then never head-of-line blocks on collectives.
  - Per layer: per-edge rows gathered via dma_gather (4 SWDGE queues),
    aggregated per 128-dst tile by matmul with an on-chip-built one-hot
    selection matrix (is_equal against iota on vector, 1/8 offloaded to
    scalar via a Relu(1-(iota-dstl)^2) trick).
  - Dense W / BN / ReLU fused per dst-tile in transposed layout; BN and
    classifier BN folded on host into per-feature scale/bias.

kernel(**inputs) -> [100000, 2] float32.
"""
import hashlib
import numpy as np
import ml_dtypes

import concourse.bacc as bacc
import concourse.bass as bass
import concourse.mybir as mybir
import concourse.tile as tile
from concourse.masks import make_identity
from concourse.bass_utils import run_bass_kernel_spmd

f32 = mybir.dt.float32
bf16 = mybir.dt.bfloat16
i16 = mybir.dt.int16
i32 = mybir.dt.int32
nbf16 = ml_dtypes.bfloat16

P = 128
NDEV = 8
EPS = 1e-5
WTAB = 128       # table row = 128 cols bf16 = 256B
NCH = 2          # allgather chunks per phase
CHT = 49         # tiles per chunk
NR = 4           # gather src ranges (2 per chunk; int16 limit 25088<=32768)
BLKT = 7         # tiles per gather block
NBLK = 14        # blocks per phase
DEFER = 4        # blocks whose chunk-1-range gathers are deferred


def _prep(x, edge_index, params, N):
    """Host preprocessing: graph partition + folded constants. Returns meta dict."""
    SHARD = N // NDEV                       # 12500
    TSHARD = ((SHARD + P - 1) // P) * P     # 12544
    NT = TSHARD // P                        # 98
    TROWS = TSHARD * NDEV                   # 100352
    CSH = CHT * P                           # 6272 rows per device per chunk
    RNGW = TROWS // NR                      # 25088 rows per range
    assert NT == NCH * CHT == NBLK * BLKT and RNGW <= 32768

    ei = edge_index.astype(np.int64)
    loop = np.arange(N, dtype=np.int64)
    dst_all = np.concatenate([ei[1], loop])
    deg = np.bincount(dst_all, minlength=N).astype(np.float32)
    dis = (1.0 / np.sqrt(deg)).astype(np.float32)

    def padrow(n):
        # node -> chunk-major padded row: chunk c spans all devices' c-th
        # 49-tile sub-shard. row = c*CSH*NDEV + d*CSH + (local - c*CSH)
        d = n // SHARD
        local = n - d * SHARD
        c = local // CSH
        return c * CSH * NDEV + d * CSH + (local - c * CSH)

    src_e = ei[0]
    dst_e = ei[1]
    psrc_e = padrow(src_e)

    counts = np.zeros((NDEV, NT, NR), dtype=np.int64)
    dev_edges = []
    for d in range(NDEV):
        m = (dst_e >= d * SHARD) & (dst_e < (d + 1) * SHARD)
        es = psrc_e[m]
        el = dst_e[m] - d * SHARD
        t_id = el >> 7
        r_id = es // RNGW
        order = np.lexsort((es, r_id, t_id))  # (tile, range, src-ascending)
        es, el, t_id, r_id = es[order], el[order], t_id[order], r_id[order]
        np.add.at(counts[d], (t_id, r_id), 1)
        dev_edges.append((es, el))

    grp_rows = counts.max(axis=0).astype(np.int64)  # [NT, NR] exact max

    grp_off = np.zeros((NT, NR), dtype=np.int64)
    blk_off = np.zeros((NBLK, NR), dtype=np.int64)
    blk_rows = np.zeros((NBLK, NR), dtype=np.int64)
    acc = 0
    for b in range(NBLK):
        for r in range(NR):
            blk_off[b, r] = acc
            for t in range(b * BLKT, (b + 1) * BLKT):
                grp_off[t, r] = acc
                acc += grp_rows[t, r]
            acc = (acc + P - 1) // P * P  # pad gather to whole chunks
            blk_rows[b, r] = acc - blk_off[b, r]
    TOT = acc
    assert TOT % 16 == 0

    # chunk-use enumeration: per (t, r) the 128-row chunks its group overlaps.
    uses = [[[] for _ in range(NR)] for _ in range(NT)]  # (local_col, dstl_col)
    n_uses = 0
    for b in range(NBLK):
        for r in range(NR):
            for t in range(b * BLKT, (b + 1) * BLKT):
                g0, g1 = grp_off[t, r], grp_off[t, r] + grp_rows[t, r]
                if g1 == g0:
                    continue
                c0, c1 = int(g0 // P), int((g1 + P - 1) // P)
                for ci in range(c0, c1):
                    uses[t][r].append((ci - int(blk_off[b, r]) // P, n_uses, ci))
                    n_uses += 1

    idx_w = np.zeros((NDEV, P, TOT // 16), dtype=np.int16)
    dstl_w = np.full((NDEV, P, n_uses), 255.0, dtype=np.float32)
    for d in range(NDEV):
        es, el = dev_edges[d]
        IDX = np.zeros(TOT, dtype=np.int16)
        DLOC = np.full(TOT, 255.0, dtype=np.float32)
        OWNER = np.full(TOT, -1, dtype=np.int64)
        pos = 0
        for t in range(NT):
            for r in range(NR):
                c = int(counts[d, t, r])
                o = int(grp_off[t, r])
                IDX[o:o + c] = (es[pos:pos + c] - r * RNGW).astype(np.int16)
                DLOC[o:o + c] = (el[pos:pos + c] - t * P).astype(np.float32)
                OWNER[o:o + int(grp_rows[t, r])] = t
                pos += c
        idx_w[d] = np.tile(IDX.reshape(-1, 16).T, (8, 1))
        D = dstl_w[d]
        for t in range(NT):
            for r in range(NR):
                for (_lc, du, ci) in uses[t][r]:
                    rows = np.arange(ci * P, (ci + 1) * P)
                    v = np.where(OWNER[rows] == t, DLOC[rows], 255.0)
                    D[:, du] = v

    dis_pad = np.zeros(TROWS, dtype=np.float32)
    dis_pad[padrow(np.arange(N))] = dis
    dis_t = np.zeros((NDEV, P, NT), dtype=np.float32)
    for d in range(NDEV):
        for t in range(NT):
            c = t // CHT
            g0 = c * CSH * NDEV + d * CSH + (t - c * CHT) * P
            dis_t[d, :, t] = dis_pad[g0:g0 + P]

    # x-tilde table (bf16, padded cols), chunk-major row space
    xt = np.zeros((TROWS, WTAB), dtype=nbf16)
    v = (dis[:, None] * x).astype(nbf16)
    xt[padrow(np.arange(N)), :x.shape[1]] = v

    def fold(g, be, rm, rv, b):
        k = (1.0 / np.sqrt(rv + EPS)).astype(np.float32)
        s = g * k
        t = (b - rm) * s + be
        return s.astype(np.float32), t.astype(np.float32)

    s1, t1 = fold(params["g1"], params["be1"], params["rm1"], params["rv1"], params["b1"])
    s2, t2 = fold(params["g2"], params["be2"], params["rm2"], params["rv2"], params["b2"])
    s3, t3 = fold(params["g3"], params["be3"], params["rm3"], params["rv3"], params["b3"])
    s4, t4 = fold(params["g4"], params["be4"], params["rm4"], params["rv4"], params["b4"])
    zk = (1.0 / np.sqrt(params["crv1"] + EPS)).astype(np.float32)
    cs1 = params["cg1"] * zk
    ct1 = -params["crm1"] * cs1 + params["cbe1"]
    zk = (1.0 / np.sqrt(params["crv2"] + EPS)).astype(np.float32)
    cs2 = params["cg2"] * zk
    ct2 = -params["crm2"] * cs2 + params["cbe2"]
    cW2p = (cs1[:, None] * params["cW2"]).astype(np.float32)
    cb2p = (ct1 @ params["cW2"] + params["cb2"]).astype(np.float32)
    cW3p = (cs2[:, None] * params["cW3"]).astype(np.float32)
    cb3p = (ct2 @ params["cW3"] + params["cb3"]).astype(np.float32)

    vecs = np.zeros((P, 13), dtype=np.float32)
    vecs[:, 0], vecs[:, 1] = s1, t1
    vecs[:, 2], vecs[:, 3] = s2[:128], t2[:128]
    vecs[:, 4], vecs[:, 5] = s2[128:], t2[128:]
    vecs[:, 6], vecs[:, 7] = s3, t3
    vecs[:64, 8], vecs[:64, 9] = s4, t4
    vecs[:64, 10] = params["cb1"]
    vecs[:32, 11] = cb2p
    vecs[:2, 12] = cb3p

    return dict(
        N=N, SHARD=SHARD, TSHARD=TSHARD, NT=NT, TROWS=TROWS, CSH=CSH, RNGW=RNGW,
        TOT=TOT, uses=uses, n_uses=n_uses,
        blk_off=blk_off, blk_rows=blk_rows,
        idx_w=idx_w, dstl_w=dstl_w, dis_t=dis_t, xt=xt, vecs=vecs,
        W1=params["W1"].astype(np.float32), W2=params["W2"].astype(np.float32),
        W3=np.concatenate([params["W3"][:128], params["W3"][128:]], axis=1).astype(np.float32),
        W4=params["W4"].astype(np.float32),
        cW1=params["cW1"].astype(np.float32), cW2p=cW2p, cW3p=cW3p,
        d_in=x.shape[1],
    )


def _build(meta):
    """Build the Bass program (same for all cores)."""
    NT, TSHARD, TROWS = meta["NT"], meta["TSHARD"], meta["TROWS"]
    CSH, RNGW, TOT = meta["CSH"], meta["RNGW"], meta["TOT"]
    uses, n_uses = meta["uses"], meta["n_uses"]
    blk_off, blk_rows = meta["blk_off"], meta["blk_rows"]
    D_IN = meta["d_in"]
    CRNG = CSH * NDEV  # rows per chunk tensor (50176) = 2 ranges

    nc = bacc.Bacc(None, target_bir_lowering=False, num_swdge_queues=4)
    t_xt = [nc.dram_tensor(f"xt{c}", [CRNG, WTAB], bf16, kind="ExternalInput")
            for c in range(NCH)]
    t_idx = nc.dram_tensor("idx", [P, TOT // 16], i16, kind="ExternalInput")
    t_dstl = nc.dram_tensor("dstl", [P, n_uses], f32, kind="ExternalInput")
    t_xto = nc.dram_tensor("xt_own", [TSHARD, WTAB], bf16, kind="ExternalInput")
    t_dis = nc.dram_tensor("dis", [P, NT], f32, kind="ExternalInput")
    t_vecs = nc.dram_tensor("vecs", [P, 13], f32, kind="ExternalInput")
    t_W1 = nc.dram_tensor("W1", [D_IN, 128], f32, kind="ExternalInput")
    t_W2 = nc.dram_tensor("W2", [128, 256], f32, kind="ExternalInput")
    t_W3 = nc.dram_tensor("W3", [128, 256], f32, kind="ExternalInput")  # packed K-halves
    t_W4 = nc.dram_tensor("W4", [128, 64], f32, kind="ExternalInput")
    t_cW1 = nc.dram_tensor("cW1", [64, 64], f32, kind="ExternalInput")
    t_cW2 = nc.dram_tensor("cW2p", [64, 32], f32, kind="ExternalInput")
    t_cW3 = nc.dram_tensor("cW3p", [32, 2], f32, kind="ExternalInput")
    t_out = nc.dram_tensor("outT", [2, TSHARD], f32, kind="ExternalOutput")

    cc_in = [[nc.dram_tensor(f"cc_in{k}_{c}", [CSH, WTAB], bf16) for c in range(NCH)]
             for k in range(3)]
    tabs = [[nc.dram_tensor(f"tab{k}_{c}", [CRNG, WTAB], bf16, addr_space="Shared")
             for c in range(NCH)] for k in range(3)]

    qctr = [0]

    def qrr():
        qctr[0] = (qctr[0] + 1) % 4
        return qctr[0]

    with tile.TileContext(nc) as tc:
        with (
            tc.tile_pool(name="const", bufs=1) as cpool,
            tc.tile_pool(name="gp", bufs=15) as gpool,
            tc.tile_pool(name="sp", bufs=6) as spool,
            tc.tile_pool(name="yq", bufs=2) as ypool,
            tc.tile_pool(name="pagg", bufs=2, space="PSUM") as pagg,
            tc.tile_pool(name="paux", bufs=3, space="PSUM") as paux,
            tc.tile_pool(name="ep", bufs=3) as ep,
        ):
            # ---- constants
            idx_sb = cpool.tile([P, TOT // 16], i16)
            nc.sync.dma_start(out=idx_sb[:], in_=t_idx[:])
            dstlf_sb = cpool.tile([P, n_uses], f32)
            nc.sync.dma_start(out=dstlf_sb[:], in_=t_dstl[:])
            dstl_sb = cpool.tile([P, n_uses], bf16)
            nc.vector.tensor_copy(out=dstl_sb[:], in_=dstlf_sb[:])
            dstln_sb = cpool.tile([P, n_uses], f32)
            nc.vector.tensor_scalar_mul(dstln_sb[:], dstlf_sb[:], -1.0)
            dis_sb = cpool.tile([P, NT], f32)
            nc.sync.dma_start(out=dis_sb[:], in_=t_dis[:])
            vecs_sb = cpool.tile([P, 13], f32)
            nc.sync.dma_start(out=vecs_sb[:], in_=t_vecs[:])
            W1_sb = cpool.tile([D_IN, 128], f32)
            nc.sync.dma_start(out=W1_sb[:], in_=t_W1[:])
            W2_sb = cpool.tile([128, 256], f32)
            nc.sync.dma_start(out=W2_sb[:], in_=t_W2[:])
            W3_sb = cpool.tile([128, 256], f32)
            nc.sync.dma_start(out=W3_sb[:], in_=t_W3[:])
            W4_sb = cpool.tile([128, 64], f32)
            nc.sync.dma_start(out=W4_sb[:], in_=t_W4[:])
            cW1_sb = cpool.tile([64, 64], f32)
            nc.sync.dma_start(out=cW1_sb[:], in_=t_cW1[:])
            cW2_sb = cpool.tile([64, 32], f32)
            nc.sync.dma_start(out=cW2_sb[:], in_=t_cW2[:])
            cW3_sb = cpool.tile([32, 2], f32)
            nc.sync.dma_start(out=cW3_sb[:], in_=t_cW3[:])
            ident = cpool.tile([P, P], f32)
            make_identity(nc, ident[:])
            ident_bf = cpool.tile([P, P], bf16)
            nc.vector.tensor_copy(out=ident_bf[:], in_=ident[:])
            KMAX = max((len(uses[t][r]) for t in range(NT) for r in range(NR)),
                       default=1)
            iota_i = cpool.tile([P, KMAX, P], i32)
            nc.gpsimd.iota(iota_i[:], pattern=[[0, KMAX], [1, P]], base=0,
                           channel_multiplier=0)
            iota_bf = cpool.tile([P, KMAX, P], bf16)
            nc.vector.tensor_copy(out=iota_bf[:], in_=iota_i[:])

            AluEq = mybir.AluOpType.is_equal
            ACTF = mybir.ActivationFunctionType

            def transpose_f32(src_sb, pdim, fdim):
                """[pdim, fdim] f32 sbuf -> [fdim, pdim] f32 sbuf (PE transpose)."""
                tp = paux.tile([fdim, pdim], f32, tag="mm")
                nc.tensor.transpose(tp[:], src_sb[:], ident[:pdim, :pdim])
                out = ep.tile([fdim, pdim], f32, tag="tps")
                nc.vector.tensor_copy(out=out[:], in_=tp[:])
                return out

            def emit_ag(k, c):
                nc.gpsimd.collective_compute(
                    "AllGather", mybir.AluOpType.bypass,
                    replica_groups=[list(range(NDEV))],
                    ins=[cc_in[k][c][:]], outs=[tabs[k][c][:]],
                )

            def emit_gather(k, b, r):
                rows = int(blk_rows[b, r])
                if rows == 0:
                    return None
                g = gpool.tile([P, rows // P, WTAB], bf16, tag="g")
                off = int(blk_off[b, r])
                if k == 0:
                    table = t_xt[r // 2]
                else:
                    table = tabs[k - 1][r // 2]
                lo = (r % 2) * RNGW
                nc.gpsimd.dma_gather(
                    out_ap=g[:],
                    in_ap=table[lo:lo + RNGW, :],
                    idxs_ap=idx_sb[:, off // 16:(off + rows) // 16],
                    num_idxs=rows,
                    num_idxs_reg=rows,
                    elem_size=WTAB,
                    single_packet=False,
                    queue_num=qrr(),
                )
                return g

            def emit_tiles(k, b, gt, w, epilogue):
                """Aggregation matmuls + epilogue for block b (gt: r->gather)."""
                for t in range(b * BLKT, (b + 1) * BLKT):
                    own = ep.tile([P, WTAB], bf16, tag="own")
                    if k == 0:
                        src_ap = t_xto[t * P:(t + 1) * P, :]
                    else:
                        c, pt = t // CHT, t % CHT
                        src_ap = cc_in[k - 1][c][pt * P:(pt + 1) * P, :]
                    nc.sync.dma_start(out=own[:], in_=src_ap)
                    spt = {}
                    for r in range(NR):
                        ul = uses[t][r]
                        if not ul:
                            continue
                        du0 = ul[0][1]
                        sP = spool.tile([P, len(ul), P], bf16, tag="s")
                        if (t * NR + r) % 8 != 7:
                            nc.vector.tensor_tensor(
                                out=sP[:],
                                in0=dstl_sb[:, du0:du0 + len(ul)].to_broadcast([P, len(ul), P]),
                                in1=iota_bf[:, :len(ul), :],
                                op=AluEq,
                            )
                        else:
                            # ACT path: s = Relu(1 - (iota - dstl)^2)
                            yq = ypool.tile([P, len(ul), P], bf16, tag="yq")
                            for ui in range(len(ul)):
                                nc.scalar.activation(
                                    yq[:, ui, :], iota_bf[:, ui, :], ACTF.Square,
                                    bias=dstln_sb[:, du0 + ui:du0 + ui + 1])
                            nc.scalar.activation(sP[:], yq[:], ACTF.Relu,
                                                 bias=1.0, scale=-1.0)
                        spt[r] = sP
                    nmm = 1 + sum(len(uses[t][r]) for r in range(NR))
                    ps = pagg.tile([P, w], f32, tag="pagg")
                    nc.tensor.matmul(ps[:], lhsT=ident_bf[:], rhs=own[:, :w],
                                     start=True, stop=(nmm == 1))
                    kk = 1
                    for r in range(NR):
                        for ui, (lc, du, _ci) in enumerate(uses[t][r]):
                            nc.tensor.matmul(
                                ps[:], lhsT=spt[r][:, ui, :], rhs=gt[r][:, lc, :w],
                                start=False, stop=(kk == nmm - 1),
                            )
                            kk += 1
                    epilogue(t, ps)

            def phase(k, w, epilogue):
                """Emission schedule (see module docstring): chunk-1-range
                gathers of blocks 0..DEFER-1 are deferred to iterations
                DEFER..2*DEFER-1; AG[k-1][1] is emitted at iteration 1;
                AG[k][0] mid-phase once chunk-0 tiles are done."""
                defer = 0 if k == 0 else DEFER  # phase 0 reads inputs: no AG to dodge
                gts = {}  # b -> {r: gather tile}
                for i in range(NBLK):
                    b = i
                    gts.setdefault(b, {})
                    if k > 0 and i == 3:
                        emit_ag(k - 1, 1)
                    # chunk-0 ranges of block b
                    for r in (0, 1):
                        gts[b][r] = emit_gather(k, b, r)
                    if b >= defer:
                        for r in (2, 3):
                            gts[b][r] = emit_gather(k, b, r)
                    # catch-up: deferred chunk-1 ranges of early blocks
                    if defer and defer <= i < 2 * defer:
                        bd = i - defer
                        for r in (2, 3):
                            gts[bd][r] = emit_gather(k, bd, r)
                        emit_tiles(k, bd, gts.pop(bd), w, epilogue)
                    if b >= defer:
                        emit_tiles(k, b, gts.pop(b), w, epilogue)
                    if k < 3 and i == 9:
                        emit_ag(k, 0)  # chunk-0 tiles (b0..6) done by now

            def wr_cc(k, t, src):
                c, pt = t // CHT, t % CHT
                nc.sync.dma_start(out=cc_in[k][c][pt * P:(pt + 1) * P, :], in_=src)

            # ================= Phase 1: L1 =================
            def ep1(t, ps):
                a = ep.tile([P, D_IN], f32, tag="a1")
                nc.scalar.activation(a[:], ps[:], ACTF.Copy, scale=dis_sb[:, t:t + 1])
                aT = transpose_f32(a, P, D_IN)
                hps = paux.tile([128, P], f32, tag="mm")
                nc.tensor.matmul(hps[:], lhsT=W1_sb[:], rhs=aT[:], start=True, stop=True)
                hT = ep.tile([128, P], f32, tag="h1T")
                nc.scalar.activation(hT[:], hps[:], ACTF.Relu,
                                     bias=vecs_sb[:, 1:2], scale=vecs_sb[:, 0:1])
                hp = paux.tile([P, 128], f32, tag="mm")
                nc.tensor.transpose(hp[:], hT[:], ident[:])
                hb = ep.tile([P, WTAB], bf16, tag="h1b")
                nc.scalar.activation(hb[:], hp[:], ACTF.Copy, scale=dis_sb[:, t:t + 1])
                wr_cc(0, t, hb[:])

            phase(0, D_IN, ep1)

            # ================= Phase 2: L2 + dense L3 =================
            def ep2(t, ps):
                a = ep.tile([P, 128], f32, tag="a2")
                nc.scalar.activation(a[:], ps[:], ACTF.Copy, scale=dis_sb[:, t:t + 1])
                aT = transpose_f32(a, P, 128)
                y3ps = paux.tile([128, P], f32, tag="acc")
                for h in range(2):
                    hps = paux.tile([128, P], f32, tag="mm")
                    nc.tensor.matmul(hps[:], lhsT=W2_sb[:, h * 128:(h + 1) * 128],
                                     rhs=aT[:], start=True, stop=True)
                    hT = ep.tile([128, P], f32, tag="h2T")
                    nc.scalar.activation(hT[:], hps[:], ACTF.Relu,
                                         bias=vecs_sb[:, 3 + 2 * h:4 + 2 * h],
                                         scale=vecs_sb[:, 2 + 2 * h:3 + 2 * h])
                    nc.tensor.matmul(y3ps[:], lhsT=W3_sb[:, h * 128:(h + 1) * 128],
                                     rhs=hT[:], start=(h == 0), stop=(h == 1))
                y3T = ep.tile([128, P], f32, tag="y3T")
                nc.vector.tensor_copy(out=y3T[:], in_=y3ps[:])
                y3p = paux.tile([P, 128], f32, tag="mm")
                nc.tensor.transpose(y3p[:], y3T[:], ident[:])
                y3b = ep.tile([P, WTAB], bf16, tag="y3b")
                nc.scalar.activation(y3b[:], y3p[:], ACTF.Copy, scale=dis_sb[:, t:t + 1])
                wr_cc(1, t, y3b[:])

            phase(1, 128, ep2)

            # ================= Phase 3: L3 agg + dense L4 =================
            def ep3(t, ps):
                z = ep.tile([P, 128], f32, tag="z3")
                nc.scalar.activation(z[:], ps[:], ACTF.Copy, scale=dis_sb[:, t:t + 1])
                zT = transpose_f32(z, P, 128)
                h3T = ep.tile([128, P], f32, tag="h3T")
                nc.scalar.activation(h3T[:], zT[:], ACTF.Relu,
                                     bias=vecs_sb[:, 7:8], scale=vecs_sb[:, 6:7])
                y4ps = paux.tile([64, P], f32, tag="mm")
                nc.tensor.matmul(y4ps[:], lhsT=W4_sb[:], rhs=h3T[:], start=True, stop=True)
                y4T = ep.tile([64, P], f32, tag="y4T")
                nc.vector.tensor_copy(out=y4T[:], in_=y4ps[:])
                y4p = paux.tile([P, 64], f32, tag="mm")
                nc.tensor.transpose(y4p[:], y4T[:], ident[:64, :64])
                y4b = ep.tile([P, WTAB], bf16, tag="y4b")
                nc.vector.memset(y4b[:, 64:], 0)
                nc.scalar.activation(y4b[:, :64], y4p[:], ACTF.Copy,
                                     scale=dis_sb[:, t:t + 1])
                wr_cc(2, t, y4b[:])

            phase(2, 128, ep3)

            # ================= Phase 4: L4 agg + classifier =================
            def ep4(t, ps):
                z = ep.tile([P, 64], f32, tag="z4")
                nc.scalar.activation(z[:], ps[:], ACTF.Copy, scale=dis_sb[:, t:t + 1])
                zT = transpose_f32(z, P, 64)
                h4T = ep.tile([64, P], f32, tag="h4T")
                nc.scalar.activation(h4T[:], zT[:], ACTF.Relu,
                                     bias=vecs_sb[:64, 9:10], scale=vecs_sb[:64, 8:9])
                u1ps = paux.tile([64, P], f32, tag="mm")
                nc.tensor.matmul(u1ps[:], lhsT=cW1_sb[:], rhs=h4T[:], start=True, stop=True)
                u1T = ep.tile([64, P], f32, tag="u1T")
                nc.scalar.activation(u1T[:], u1ps[:], ACTF.Relu, bias=vecs_sb[:64, 10:11])
                u2ps = paux.tile([32, P], f32, tag="mm")
                nc.tensor.matmul(u2ps[:], lhsT=cW2_sb[:], rhs=u1T[:], start=True, stop=True)
                u2T = ep.tile([32, P], f32, tag="u2T")
                nc.scalar.activation(u2T[:], u2ps[:], ACTF.Relu, bias=vecs_sb[:32, 11:12])
                ops_ = paux.tile([2, P], f32, tag="mm")
                nc.tensor.matmul(ops_[:], lhsT=cW3_sb[:], rhs=u2T[:], start=True, stop=True)
                oT = ep.tile([2, P], f32, tag="oT")
                nc.scalar.activation(oT[:], ops_[:], ACTF.Identity, bias=vecs_sb[:2, 12:13])
                nc.sync.dma_start(out=t_out[:, t * P:(t + 1) * P], in_=oT[:])

            phase(3, 64, ep4)

    nc.finalize()
    return nc


_CACHE = {}


def kernel(**inputs):
    x = np.asarray(inputs["x"], dtype=np.float32)
    edge_index = np.asarray(inputs["edge_index"])
    N = x.shape[0]
    key = hashlib.sha256(edge_index.tobytes()).hexdigest()[:16] + f"_{N}_{x.shape[1]}"
    if key not in _CACHE:
        meta = _prep(x, edge_index, inputs, N)
        nc = _build(meta)
        _CACHE[key] = (meta, nc)
    else:
        meta, nc = _CACHE[key]
        # x may differ between calls with same graph: recompute xt
        meta = dict(meta)
        m2 = _prep(x, edge_index, inputs, N)
        meta["xt"] = m2["xt"]
        meta.update({k: m2[k] for k in ("vecs", "W1", "W2", "W3", "W4", "cW1", "cW2p", "cW3p", "dis_t")})

    CSH = meta["CSH"]
    CRNG = CSH * NDEV
    in_maps = []
    for d in range(NDEV):
        im = {
            "idx": meta["idx_w"][d],
            "dstl": meta["dstl_w"][d],
            "dis": meta["dis_t"][d],
            "vecs": meta["vecs"],
            "W1": meta["W1"], "W2": meta["W2"], "W3": meta["W3"], "W4": meta["W4"],
            "cW1": meta["cW1"], "cW2p": meta["cW2p"], "cW3p": meta["cW3p"],
        }
        for c in range(NCH):
            im[f"xt{c}"] = meta["xt"][c * CRNG:(c + 1) * CRNG]
        im["xt_own"] = np.concatenate([
            meta["xt"][c * CRNG + d * CSH:c * CRNG + (d + 1) * CSH]
            for c in range(NCH)
        ])
        in_maps.append(im)
    res = None
    for _attempt in range(4):
        try:
            res = run_bass_kernel_spmd(nc, in_maps, core_ids=list(range(NDEV)), trace=False)
            break
        except Exception:
            if _attempt == 3:
                raise

    SHARD = meta["SHARD"]
    out = np.empty((N, 2), dtype=np.float32)
    for d in range(NDEV):
        out[d * SHARD:(d + 1) * SHARD] = res.results[d]["outT"][:, :SHARD].T
    return out
